# revision 7
# baseline (speedup 1.0000x reference)
"""Multi-head causal self-attention on 8 Trainium2 NeuronCores.

Problem: x [4, 2048, 1024], Wq/Wk/Wv/Wo [1024, 1024] (applied as x @ W.T),
16 heads, dk=64, causal softmax, output [4, 2048, 1024], all fp32.

Sharding: 8 cores = 4 batches x 2 head-groups (8 heads each).
Each core computes QKV projections for its 8 heads, streaming causal
attention, and a partial output projection (Wo row-split). The host adds
the two partial outputs per batch element.

Per-core layouts (chosen so NO on-device transposes are needed):
  xT  [1024, 2048]  = x[b].T          (host-transposed)
  wqT [1024, 512]   = (Wq/8).T cols for this head group (1/sqrt(dk) folded)
  wkT [1024, 512], wvT [1024, 512]
  woT [512, 1024]   = Wo[:, cols].T
  QT/KT on chip as [feat, seq] (head pairs stacked on partitions),
  V as [seq, 65*8] bf16 with a ones column appended per head, so the
  AV matmul (stationary [V | 1], 65 outputs) yields both the context
  rows AND the softmax denominator row in one pass, in the same PE
  tiling mode (128x128) as the projection fills. Scores tiles
  [k=128, q=512] per head pair are exp'ed on ScalarE into bf16; the
  causal mask is applied with affine_select on the idle GpSimd engine;
  1/l is broadcast across partitions with gpsimd partition_broadcast.

Causal-wedge (diagonal) tiles are narrowed to their valid q-range, and
projection s-chunks / output-projection blocks are interleaved in
program order so TensorE, ScalarE and GpSimd work concurrently.
"""

import ml_dtypes
import numpy as np

import concourse.bass as bass
import concourse.mybir as mybir
import concourse.tile as tile
from concourse.bass_utils import run_bass_kernel_spmd
from concourse.vector_clock import ScopedClock

F32 = mybir.dt.float32
F32R = mybir.dt.float32r
BF16 = mybir.dt.bfloat16
AF = mybir.ActivationFunctionType
ALU = mybir.AluOpType

B, S, D = 4, 2048, 1024
H = 16
DK = 64
N_CORES = 8
HG = 512          # head-group width (8 heads x 64)


# ---------------------------------------------------------------------------
# This walrus accepts at most 1 sem wait per instruction (2 for
# EventSemaphore). Tile emits more in two places; both are fixed up here by
# moving excess waits onto preceding instructions on the same engine.
# ---------------------------------------------------------------------------
def _split_drain_and_barrier(self, tick_clock, wait_clock):
    nc = self.nc
    probe = nc.sync.nop(nofuse=True, hint="tile_drain_waits")
    wait_clock.add_sem_waits(
        probe.ins, ScopedClock({None: tick_clock.global_clock})
    )
    si = probe.ins.sync_info
    waits = list(si.on_wait) if si is not None else []
    if len(waits) > 1:
        probe.ins.sync_info = mybir.SyncInfo(on_wait=[waits[0]], on_update=[])
        for w in waits[1:]:
            n = nc.sync.nop(nofuse=True, hint="tile_drain_waits")
            n.ins.sync_info = mybir.SyncInfo(on_wait=[w], on_update=[])
    nc.sync.drain()
    nc.all_engine_barrier()
    popped = nc._tile_sem_poison_stack.pop()
    assert popped is self._sem_poison
    nc.clear_and_free_semaphores(list(self.sems.allocated().values()))
    nc.all_engine_barrier()


tile.TileContext._drain_and_barrier = _split_drain_and_barrier

_wsplit_counter = [0]


def _enforce_wait_limits(m):
    for fn in m.functions:
        for bb in fn.blocks:
            out = []
            changed = False
            for inst in bb.instructions:
                si = inst.sync_info
                cap = 2 if isinstance(inst, mybir.InstEventSemaphore) else 1
                if si is not None and len(si.on_wait) > cap:
                    waits = list(si.on_wait)
                    keep, extra = waits[:cap], waits[cap:]
                    for i in range(0, len(extra), 2):
                        _wsplit_counter[0] += 1
                        out.append(mybir.InstEventSemaphore(
                            name=f"I-wsplit-{_wsplit_counter[0]}",
                            engine=inst.engine,
                            ins=[], outs=[],
                            sync_info=mybir.SyncInfo(
                                on_wait=extra[i:i + 2], on_update=[]),
                        ))
                    inst.sync_info = mybir.SyncInfo(
                        on_wait=keep, on_update=list(si.on_update))
                    changed = True
                out.append(inst)
            if changed:
                bb.instructions = out


def build_nc():
    nc = bass.Bass()

    xT = nc.declare_dram_parameter("xT", [D, S], BF16, isOutput=False)
    wqT = nc.declare_dram_parameter("wqT", [D, HG], BF16, isOutput=False)
    wkT = nc.declare_dram_parameter("wkT", [D, HG], BF16, isOutput=False)
    wvT = nc.declare_dram_parameter("wvT", [D, HG], BF16, isOutput=False)
    woT = nc.declare_dram_parameter("woT", [HG, D], BF16, isOutput=False)
    yout = nc.declare_dram_parameter("y", [S, D], BF16, isOutput=True)

    KT8 = D // 128   # contraction tiles for the projections
    NP = 4           # head pairs per core
    NS = S // 128    # seq tiles of 128
    VW = 65          # per-head V width in the augmented layout ([V | 1])

    from contextlib import ExitStack

    with tile.TileContext(nc) as tc, ExitStack() as ctx:
        ep = ctx.enter_context
        consts = ep(tc.tile_pool(name="consts", bufs=1))
        qt_pool = ep(tc.tile_pool(name="qt", bufs=1))
        kt_pool = ep(tc.tile_pool(name="kt", bufs=1))
        v_pool = ep(tc.tile_pool(name="v", bufs=1))
        wo_pool = ep(tc.tile_pool(name="wo", bufs=1))
        wq_pool = ep(tc.tile_pool(name="wq", bufs=1))
        wk_pool = ep(tc.tile_pool(name="wk", bufs=1))
        wv_pool = ep(tc.tile_pool(name="wv", bufs=1))
        xt_pool = ep(tc.tile_pool(name="xt", bufs=2))
        exp_pool = ep(tc.tile_pool(name="exp", bufs=3))
        ctxn_pool = ep(tc.tile_pool(name="ctxn", bufs=12))
        rcp_pool = ep(tc.tile_pool(name="rcp", bufs=4))
        cnb_pool = ep(tc.tile_pool(name="cnb", bufs=2))
        ctxraw_pool = ep(tc.tile_pool(name="ctxraw", bufs=4))
        ybuf_pool = ep(tc.tile_pool(name="ybuf", bufs=2))
        mm_ps = ep(tc.tile_pool(name="mm_ps", bufs=2, space="PSUM"))
        sc_ps = ep(tc.tile_pool(name="sc_ps", bufs=2, space="PSUM"))
        ctx_ps = ep(tc.tile_pool(name="ctx_ps", bufs=2, space="PSUM"))

        # broadcast constant: bcA.T @ m replicates m's row 64 onto rows 0-63
        bcA = consts.tile([VW, 128], BF16, tag="bcA", name="bcA")
        nc.gpsimd.memset(bcA[:], 0.0)
        nc.gpsimd.memset(bcA[64:65, 0:64], 1.0)

        QT = [qt_pool.tile([128, S], BF16, tag=f"qt{p}", name=f"QT{p}")
              for p in range(NP)]
        KTt = [kt_pool.tile([128, S], BF16, tag=f"kt{p}", name=f"KTt{p}")
               for p in range(NP)]
        # V_aug: per seq-tile, [128, 8*65]: per head 64 V columns + a ones
        # column, so the AV stationary [V | 1] produces ctx rows 0-63 and
        # the softmax denominator at row 64.
        V = [v_pool.tile([128, 8 * VW], BF16, tag=f"v{s}", name=f"V{s}")
             for s in range(NS)]
        for s in range(NS):
            ones_ap = V[s][:].rearrange("p (h c) -> p h c", c=VW)[:, :, 64:65]
            nc.gpsimd.memset(ones_ap, 1.0)

        # ---- weight DMAs: sync ring carries weights in first-use order;
        # gpsimd ring carries the xT chunks. The first Q matmul only waits
        # for wq[0] + xt0[0], so compute starts ~1us in.
        def emit_xt_dmas(st):
            xts = []
            for kt in range(KT8):
                t = xt_pool.tile([128, 512], BF16, tag=f"xt{kt}",
                                 name=f"xt{st}_{kt}")
                nc.gpsimd.dma_start(
                    t[:], xT[kt * 128:(kt + 1) * 128, st * 512:(st + 1) * 512]
                )
                xts.append(t)
            return xts

        xts0 = emit_xt_dmas(0)
        wq_t, wk_t, wv_t, wo_t = [], [], [], []
        for kt in range(KT8):
            t = wq_pool.tile([128, HG], BF16, tag=f"w{kt}", name=f"wq{kt}")
            nc.sync.dma_start(t[:], wqT[kt * 128:(kt + 1) * 128, :])
            wq_t.append(t)

        def proj_items(st, xts):
            """QKV projection work for chunk st as a flat list of closures,
            one instruction each, so they can be sprinkled between attention
            stages at fine grain."""
            items = []

            def qk_group(ot, w_t, dst, name):
                holder = {}

                def mk_mm(kt):
                    def go():
                        if "ps" not in holder:
                            holder["ps"] = mm_ps.tile(
                                [128, 512], F32, tag="mm", name=name)
                        nc.tensor.matmul(
                            holder["ps"][:],
                            w_t[kt][:, ot * 128:(ot + 1) * 128],
                            xts[kt][:],
                            start=(kt == 0),
                            stop=(kt == KT8 - 1),
                        )
                    return go

                def copy():
                    nc.vector.tensor_copy(
                        dst[ot][:, st * 512:(st + 1) * 512], holder["ps"][:])

                return [mk_mm(kt) for kt in range(KT8)] + [copy]

            def v_group(sub):
                holder = {}

                def mk_mm(kt):
                    def go():
                        if "ps" not in holder:
                            holder["ps"] = mm_ps.tile(
                                [128, 512], F32, tag="mm", name=f"pv{st}{sub}")
                        nc.tensor.matmul(
                            holder["ps"][:],
                            xts[kt][:, sub * 128:(sub + 1) * 128],
                            wv_t[kt][:],
                            start=(kt == 0),
                            stop=(kt == KT8 - 1),
                        )
                    return go

                def copy():
                    dst = V[st * 4 + sub][:].rearrange(
                        "p (h c) -> p h c", c=VW)[:, :, 0:64]
                    src = holder["ps"][:].rearrange("p (h c) -> p h c", c=64)
                    nc.vector.tensor_copy(dst, src)

                return [mk_mm(kt) for kt in range(KT8)] + [copy]

            for ot in range(NP):
                items.extend(qk_group(ot, wq_t, QT, f"pq{st}{ot}"))
            for ot in range(NP):
                items.extend(qk_group(ot, wk_t, KTt, f"pk{st}{ot}"))
            for sub in range(4):
                items.extend(v_group(sub))
            return items

        def attention_block(j, fill, ctxn):
            """Causal attention + softmax-denominator for q-block j.
            `fill` is a list of closures (projections / output projections)
            sprinkled into the PE stream to cover exp-wait stalls. Normalized
            context tiles are appended to `ctxn` eagerly (the normalize chain
            runs on DVE+GpSimd only, so it never stalls the PE)."""
            fill = list(fill)
            n_triples = NP * 4 * (j + 1)
            per_triple = -(-len(fill) // n_triples) if fill else 0

            def emit_fill(n):
                for _ in range(n):
                    if not fill:
                        return
                    if fill[0]() is False:
                        return  # head item's inputs not produced yet
                    fill.pop(0)

            ni = 4 * (j + 1)

            def scores(pair, i):
                p = i - 4 * j
                lo = 128 * p if p > 0 else 0
                sc = sc_ps.tile([128, 1024], F32, tag="sc",
                                name=f"sc{j}{pair}{i}")
                qa = QT[pair][0:64, j * 512 + lo:(j + 1) * 512]
                qb = QT[pair][64:128, j * 512 + lo:(j + 1) * 512]
                ka = KTt[pair][0:64, i * 128:(i + 1) * 128]
                kb = KTt[pair][64:128, i * 128:(i + 1) * 128]
                nc.tensor.matmul(
                    sc[:, lo:512], ka, qa,
                    start=True, stop=True, tile_position=(0, 0),
                )
                nc.tensor.matmul(
                    sc[:, 512 + lo:1024], kb, qb,
                    start=True, stop=True, tile_position=(64, 0),
                )
                return sc

            def emit_exp(sc, i, pair):
                p = i - 4 * j
                et = exp_pool.tile([128, 1024], BF16, tag="exp",
                                   name=f"et{j}{pair}{i}")
                if p >= 2:
                    lo = 128 * p
                    nc.scalar.activation(
                        et[:, lo:512], sc[:, lo:512], AF.Exp)
                    nc.scalar.activation(
                        et[:, 512 + lo:1024], sc[:, 512 + lo:1024], AF.Exp)
                else:
                    nc.scalar.activation(et[:], sc[:], AF.Exp)
                if p >= 0:
                    # diagonal block: zero the future positions within the
                    # 128-wide triangle at [lo, lo+128): keep iff qq' >= kk
                    lo = 128 * p
                    ap = et[:].rearrange(
                        "p (h q) -> p h q", h=2)[:, :, lo:lo + 128]
                    nc.gpsimd.affine_select(
                        out=ap, in_=ap,
                        pattern=[[0, 2], [1, 128]],
                        compare_op=ALU.is_ge,
                        fill=0.0,
                        base=0,
                        channel_multiplier=-1,
                    )
                return et

            def emit_ctx(ctx_a, ctx_b, et, i):
                p = i - 4 * j
                lo = 128 * p if p > 0 else 0
                first, last = (i == 0), (i == ni - 1)
                va = V[i][:, pair * 2 * VW: pair * 2 * VW + VW]
                vb = V[i][:, pair * 2 * VW + VW: pair * 2 * VW + 2 * VW]
                nc.tensor.matmul(
                    ctx_a[:, lo:512], va, et[:, lo:512],
                    start=first, stop=last, skip_group_check=True,
                )
                nc.tensor.matmul(
                    ctx_b[:, lo:512], vb, et[:, 512 + lo:1024],
                    start=first, stop=last, skip_group_check=True,
                )

            for pair in range(NP):
                ctx_a = ctx_ps.tile([VW, 512], F32, tag="ctx",
                                    name=f"ctxa{j}{pair}")
                ctx_b = ctx_ps.tile([VW, 512], F32, tag="ctx",
                                    name=f"ctxb{j}{pair}")
                sc = scores(pair, 0)
                pending = None
                for i in range(ni):
                    et = emit_exp(sc, i, pair)
                    if pending is not None:
                        emit_ctx(ctx_a, ctx_b, *pending)
                        emit_fill(per_triple)
                    if i + 1 < ni:
                        sc = scores(pair, i + 1)
                    pending = (et, i)
                emit_ctx(ctx_a, ctx_b, *pending)
                emit_fill(per_triple)
                # drain ctx psum (incl. the l row at 64) and normalize:
                # a tiny bcA matmul replicates l onto rows 0-63, reciprocal +
                # multiply run partition-aligned on DVE, and one small DMA
                # moves head b's rows to partitions 64-127.
                with nc.allow_low_precision("bf16/fp32r attention pipeline"):
                    cra = ctxraw_pool.tile([VW, 512], BF16, tag="cr",
                                           name=f"cra{j}{pair}")
                    nc.vector.tensor_copy(cra[:], ctx_a[:])
                    crb = ctxraw_pool.tile([VW, 512], BF16, tag="cr",
                                           name=f"crb{j}{pair}")
                    nc.vector.tensor_copy(crb[:], ctx_b[:])
                    bps_a = mm_ps.tile([128, 512], F32, tag="mm",
                                       name=f"bpa{j}{pair}")
                    nc.tensor.matmul(bps_a[:], bcA[:], cra[:],
                                     start=True, stop=True)
                    bps_b = mm_ps.tile([128, 512], F32, tag="mm",
                                       name=f"bpb{j}{pair}")
                    nc.tensor.matmul(bps_b[:], bcA[:], crb[:],
                                     start=True, stop=True)
                    rca = rcp_pool.tile([64, 512], F32R, tag="rc",
                                        name=f"rca{j}{pair}")
                    nc.vector.reciprocal(rca[:], bps_a[0:64, :])
                    rcb = rcp_pool.tile([64, 512], F32R, tag="rc",
                                        name=f"rcb{j}{pair}")
                    nc.vector.reciprocal(rcb[:], bps_b[0:64, :])
                    cn = ctxn_pool.tile([128, 512], BF16, tag="cn",
                                        name=f"cn{j}{pair}")
                    nc.vector.tensor_mul(cn[0:64, :], cra[0:64, :], rca[:])
                    cnb = cnb_pool.tile([64, 512], BF16, tag="cnb",
                                        name=f"cnb{j}{pair}")
                    nc.vector.tensor_mul(cnb[:], crb[0:64, :], rcb[:])
                    nc.gpsimd.dma_start(cn[64:128, :], cnb[:])
                ctxn.append(cn)

            # drain any remaining fill (all inputs exist by block end)
            while fill:
                assert fill[0]() is not False
                fill.pop(0)

        def outproj_items(j, ctxn):
            """Output projection for q-tile j as fine-grain fill items."""
            items = []

            def group(s4, oh, holder):
                def mk_mm(pair):
                    def go():
                        if len(ctxn) <= pair:
                            return False  # cn not normalized yet
                        if "ps" not in holder:
                            holder["ps"] = mm_ps.tile(
                                [128, 512], F32, tag="mm", name=f"yp{j}{s4}{oh}")
                        nc.tensor.matmul(
                            holder["ps"][:],
                            ctxn[pair][:, s4 * 128:(s4 + 1) * 128],
                            wo_t[pair][:, oh * 512:(oh + 1) * 512],
                            start=(pair == 0),
                            stop=(pair == NP - 1),
                        )
                    return go

                def copy():
                    nc.vector.tensor_copy(
                        holder["yb"][:, oh * 512:(oh + 1) * 512], holder["ps"][:])
                    del holder["ps"]

                return [mk_mm(p) for p in range(NP)] + [copy]

            for s4 in range(4):
                srow = j * 4 + s4
                holder = {}

                def alloc_yb(holder=holder, s4=s4):
                    holder["yb"] = ybuf_pool.tile(
                        [128, D], BF16, tag="yb", name=f"yb{j}{s4}")

                items.append(alloc_yb)
                for oh in range(2):
                    items.extend(group(s4, oh, holder))

                def dma_out(holder=holder, srow=srow):
                    nc.sync.dma_start(
                        yout[srow * 128:(srow + 1) * 128, :], holder["yb"][:])

                items.append(dma_out)
            return items

        # chunk 0 projections run alone (Q first, matching weight-DMA
        # arrival order); attention block j then carries chunk j+1's
        # projections and block j-1's output projection as PE filler for its
        # exp-wait stalls; block 3 additionally self-fills with its own
        # output projection (enabled by the eager, PE-free normalize).
        items0 = proj_items(0, xts0)
        for item in items0[:4 * (KT8 + 1)]:   # Q groups
            item()
        for kt in range(KT8):
            t = wk_pool.tile([128, HG], BF16, tag=f"w{kt}", name=f"wk{kt}")
            nc.sync.dma_start(t[:], wkT[kt * 128:(kt + 1) * 128, :])
            wk_t.append(t)
        for item in items0[4 * (KT8 + 1):8 * (KT8 + 1)]:   # K groups
            item()
        for kt in range(KT8):
            t = wv_pool.tile([128, HG], BF16, tag=f"w{kt}", name=f"wv{kt}")
            nc.sync.dma_start(t[:], wvT[kt * 128:(kt + 1) * 128, :])
            wv_t.append(t)
        for item in items0[8 * (KT8 + 1):]:   # V groups
            item()
        for c in range(NP):
            t = wo_pool.tile([128, D], BF16, tag=f"wo{c}")
            nc.sync.dma_start(t[:], woT[c * 128:(c + 1) * 128, :])
            wo_t.append(t)

        prev_out = []
        for j in range(4):
            fill = list(prev_out)
            if j + 1 < 4:
                xts = emit_xt_dmas(j + 1)
                fill = proj_items(j + 1, xts) + fill
            ctxn = []
            if j == 3:
                prev_out = outproj_items(j, ctxn)
                fill = fill + prev_out
                prev_out = []
            attention_block(j, fill, ctxn)
            if j < 3:
                prev_out = outproj_items(j, ctxn)
        for item in prev_out:
            item()

    _enforce_wait_limits(nc.m)
    return nc


_NC = None


def _get_nc():
    global _NC
    if _NC is None:
        _NC = build_nc()
    return _NC


def run(x, Wq, Wk, Wv, Wo, trace=False, trace_kwargs=None):
    """Returns (y, BassKernelResults)."""
    x = np.asarray(x, np.float32)
    scale = 1.0 / np.sqrt(DK)
    in_maps = []
    for core in range(N_CORES):
        b, g = core // 2, core % 2
        cols = slice(g * HG, (g + 1) * HG)
        bf = ml_dtypes.bfloat16
        in_maps.append({
            "xT": np.ascontiguousarray(x[b].T).astype(bf),
            "wqT": np.ascontiguousarray(
                np.asarray(Wq, np.float32).T[:, cols] * scale).astype(bf),
            "wkT": np.ascontiguousarray(
                np.asarray(Wk, np.float32).T[:, cols]).astype(bf),
            "wvT": np.ascontiguousarray(
                np.asarray(Wv, np.float32).T[:, cols]).astype(bf),
            "woT": np.ascontiguousarray(
                np.asarray(Wo, np.float32).T[cols, :]).astype(bf),
        })
    kw = dict(trace_kwargs or {})
    res = run_bass_kernel_spmd(
        _get_nc(), in_maps, list(range(N_CORES)), trace=trace, **kw
    )
    y = np.empty((B, S, D), np.float32)
    for b in range(B):
        y[b] = (res.results[2 * b]["y"].astype(np.float32)
                + res.results[2 * b + 1]["y"].astype(np.float32))
    return y, res


def kernel(x, Wq, Wk, Wv, Wo):
    y, _ = run(x, Wq, Wk, Wv, Wo)
    return y


# revision 14
# speedup vs baseline: 1.1569x; 1.1569x over previous
"""Multi-head causal self-attention on 8 Trainium2 NeuronCores.

Problem: x [4, 2048, 1024], Wq/Wk/Wv/Wo [1024, 1024] (applied as x @ W.T),
16 heads, dk=64, causal softmax, output [4, 2048, 1024], all fp32.

Sharding: 8 cores = 4 batches x 2 head-groups (8 heads each).
Each core computes QKV projections for its 8 heads, streaming causal
attention, and a partial output projection (Wo row-split). The host adds
the two partial outputs per batch element.

Per-core layouts (chosen so NO on-device transposes are needed):
  xT  [1024, 2048]  = x[b].T          (host-transposed)
  wqT [1024, 512]   = (Wq/8).T cols for this head group (1/sqrt(dk) folded)
  wkT [1024, 512], wvT [1024, 512]
  woT [512, 1024]   = Wo[:, cols].T
  QT/KT on chip as [feat, seq] (head pairs stacked on partitions),
  V as [seq, 65*8] bf16 with a ones column appended per head, so the
  AV matmul (stationary [V | 1], 65 outputs) yields both the context
  rows AND the softmax denominator row in one pass, in the same PE
  tiling mode (128x128) as the projection fills. Scores tiles
  [k=128, q=512] per head pair are exp'ed on ScalarE into bf16; the
  causal mask is applied with affine_select on the idle GpSimd engine;
  1/l is broadcast across partitions with gpsimd partition_broadcast.

Causal-wedge (diagonal) tiles are narrowed to their valid q-range, and
projection s-chunks / output-projection blocks are interleaved in
program order so TensorE, ScalarE and GpSimd work concurrently.
"""

import ml_dtypes
import numpy as np

import concourse.bass as bass
import concourse.mybir as mybir
import concourse.tile as tile
from concourse.bass_utils import run_bass_kernel_spmd
from concourse.vector_clock import ScopedClock

F32 = mybir.dt.float32
F32R = mybir.dt.float32r
BF16 = mybir.dt.bfloat16
AF = mybir.ActivationFunctionType
ALU = mybir.AluOpType

B, S, D = 4, 2048, 1024
H = 16
DK = 64
N_CORES = 8
HG = 512          # head-group width (8 heads x 64)


# ---------------------------------------------------------------------------
# This walrus accepts at most 1 sem wait per instruction (2 for
# EventSemaphore). Tile emits more in two places; both are fixed up here by
# moving excess waits onto preceding instructions on the same engine.
# ---------------------------------------------------------------------------
def _split_drain_and_barrier(self, tick_clock, wait_clock):
    nc = self.nc
    probe = nc.sync.nop(nofuse=True, hint="tile_drain_waits")
    wait_clock.add_sem_waits(
        probe.ins, ScopedClock({None: tick_clock.global_clock})
    )
    si = probe.ins.sync_info
    waits = list(si.on_wait) if si is not None else []
    if len(waits) > 1:
        probe.ins.sync_info = mybir.SyncInfo(on_wait=[waits[0]], on_update=[])
        for w in waits[1:]:
            n = nc.sync.nop(nofuse=True, hint="tile_drain_waits")
            n.ins.sync_info = mybir.SyncInfo(on_wait=[w], on_update=[])
    nc.sync.drain()
    nc.all_engine_barrier()
    popped = nc._tile_sem_poison_stack.pop()
    assert popped is self._sem_poison
    nc.clear_and_free_semaphores(list(self.sems.allocated().values()))
    nc.all_engine_barrier()


tile.TileContext._drain_and_barrier = _split_drain_and_barrier

_wsplit_counter = [0]


def _enforce_wait_limits(m):
    for fn in m.functions:
        for bb in fn.blocks:
            out = []
            changed = False
            for inst in bb.instructions:
                si = inst.sync_info
                cap = 2 if isinstance(inst, mybir.InstEventSemaphore) else 1
                if si is not None and len(si.on_wait) > cap:
                    waits = list(si.on_wait)
                    keep, extra = waits[:cap], waits[cap:]
                    for i in range(0, len(extra), 2):
                        _wsplit_counter[0] += 1
                        out.append(mybir.InstEventSemaphore(
                            name=f"I-wsplit-{_wsplit_counter[0]}",
                            engine=inst.engine,
                            ins=[], outs=[],
                            sync_info=mybir.SyncInfo(
                                on_wait=extra[i:i + 2], on_update=[]),
                        ))
                    inst.sync_info = mybir.SyncInfo(
                        on_wait=keep, on_update=list(si.on_update))
                    changed = True
                out.append(inst)
            if changed:
                bb.instructions = out


def build_nc():
    nc = bass.Bass()

    xT = nc.declare_dram_parameter("xT", [D, S], BF16, isOutput=False)
    wqT = nc.declare_dram_parameter("wqT", [D, HG], BF16, isOutput=False)
    wkT = nc.declare_dram_parameter("wkT", [D, HG], BF16, isOutput=False)
    wvT = nc.declare_dram_parameter("wvT", [D, HG], BF16, isOutput=False)
    woT = nc.declare_dram_parameter("woT", [HG, D], BF16, isOutput=False)
    yout = nc.declare_dram_parameter("y", [S, D], BF16, isOutput=True)

    KT8 = D // 128   # contraction tiles for the projections
    NP = 4           # head pairs per core
    NS = S // 128    # seq tiles of 128
    VW = 65          # per-head V width in the augmented layout ([V | 1])

    from contextlib import ExitStack

    with tile.TileContext(nc) as tc, ExitStack() as ctx:
        ep = ctx.enter_context
        consts = ep(tc.tile_pool(name="consts", bufs=1))
        qt_pool = ep(tc.tile_pool(name="qt", bufs=1))
        kt_pool = ep(tc.tile_pool(name="kt", bufs=1))
        v_pool = ep(tc.tile_pool(name="v", bufs=1))
        wo_pool = ep(tc.tile_pool(name="wo", bufs=1))
        wq_pool = ep(tc.tile_pool(name="wq", bufs=1))
        wk_pool = ep(tc.tile_pool(name="wk", bufs=1))
        wv_pool = ep(tc.tile_pool(name="wv", bufs=1))
        xt_pool = ep(tc.tile_pool(name="xt", bufs=2))
        exp_pool = ep(tc.tile_pool(name="exp", bufs=3))
        ctxn_pool = ep(tc.tile_pool(name="ctxn", bufs=12))
        cnb_pool = ep(tc.tile_pool(name="cnb", bufs=2))
        ctxraw_pool = ep(tc.tile_pool(name="ctxraw", bufs=4))
        ybuf_pool = ep(tc.tile_pool(name="ybuf", bufs=2))
        mm_ps = ep(tc.tile_pool(name="mm_ps", bufs=2, space="PSUM"))
        sc_ps = ep(tc.tile_pool(name="sc_ps", bufs=2, space="PSUM"))
        ctx_ps = ep(tc.tile_pool(name="ctx_ps", bufs=2, space="PSUM"))

        # broadcast constant: bcA.T @ m replicates m's row 64 onto rows 0-63
        bcA = consts.tile([VW, 128], BF16, tag="bcA", name="bcA")
        nc.gpsimd.memset(bcA[:], 0.0)
        nc.gpsimd.memset(bcA[64:65, 0:64], 1.0)
        # static scratch rows for 1/l (row 64 live, rows 0-63 stay zero)
        rec_t, rec_bf = [], []
        for i in range(4):
            t = consts.tile([VW, 512], F32, tag=f"rec{i}", name=f"rec{i}")
            nc.gpsimd.memset(t[64:65, :], 0.0)
            rec_t.append(t)
            tb = consts.tile([VW, 512], BF16, tag=f"recb{i}", name=f"recb{i}")
            nc.gpsimd.memset(tb[:], 0.0)
            rec_bf.append(tb)

        QT = [qt_pool.tile([128, S], BF16, tag=f"qt{p}", name=f"QT{p}")
              for p in range(NP)]
        KTt = [kt_pool.tile([128, S], BF16, tag=f"kt{p}", name=f"KTt{p}")
               for p in range(NP)]
        # V_aug: per seq-tile, [128, 8*65]: per head 64 V columns + a ones
        # column, so the AV stationary [V | 1] produces ctx rows 0-63 and
        # the softmax denominator at row 64.
        V = [v_pool.tile([128, 8 * VW], BF16, tag=f"v{s}", name=f"V{s}")
             for s in range(NS)]
        for s in range(NS):
            ones_ap = V[s][:].rearrange("p (h c) -> p h c", c=VW)[:, :, 64:65]
            nc.gpsimd.memset(ones_ap, 1.0)

        # ---- weight DMAs: sync ring carries weights in first-use order;
        # gpsimd ring carries the xT chunks. The first Q matmul only waits
        # for wq[0] + xt0[0], so compute starts ~1us in.
        def emit_xt_dmas(st):
            xts = []
            for kt in range(KT8):
                t = xt_pool.tile([128, 512], BF16, tag=f"xt{kt}",
                                 name=f"xt{st}_{kt}")
                nc.gpsimd.dma_start(
                    t[:], xT[kt * 128:(kt + 1) * 128, st * 512:(st + 1) * 512]
                )
                xts.append(t)
            return xts

        xts0 = emit_xt_dmas(0)
        wq_t, wk_t, wv_t, wo_t = [], [], [], []
        for kt in range(KT8):
            t = wq_pool.tile([128, HG], BF16, tag=f"w{kt}", name=f"wq{kt}")
            nc.sync.dma_start(t[:], wqT[kt * 128:(kt + 1) * 128, :])
            wq_t.append(t)

        def proj_items(st, xts):
            """QKV projection work for chunk st as a flat list of closures,
            one instruction each, so they can be sprinkled between attention
            stages at fine grain."""
            items = []

            def qk_group(ot, w_t, dst, name):
                holder = {}

                def mk_mm(kt):
                    def go():
                        if "ps" not in holder:
                            holder["ps"] = mm_ps.tile(
                                [128, 512], F32, tag="mm", name=name)
                        nc.tensor.matmul(
                            holder["ps"][:],
                            w_t[kt][:, ot * 128:(ot + 1) * 128],
                            xts[kt][:],
                            start=(kt == 0),
                            stop=(kt == KT8 - 1),
                        )
                    return go

                def copy():
                    nc.vector.tensor_copy(
                        dst[ot][:, st * 512:(st + 1) * 512], holder["ps"][:])

                return [mk_mm(kt) for kt in range(KT8)] + [copy]

            def v_group(sub):
                holder = {}

                def mk_mm(kt):
                    def go():
                        if "ps" not in holder:
                            holder["ps"] = mm_ps.tile(
                                [128, 512], F32, tag="mm", name=f"pv{st}{sub}")
                        nc.tensor.matmul(
                            holder["ps"][:],
                            xts[kt][:, sub * 128:(sub + 1) * 128],
                            wv_t[kt][:],
                            start=(kt == 0),
                            stop=(kt == KT8 - 1),
                        )
                    return go

                def copy():
                    dst = V[st * 4 + sub][:].rearrange(
                        "p (h c) -> p h c", c=VW)[:, :, 0:64]
                    src = holder["ps"][:].rearrange("p (h c) -> p h c", c=64)
                    nc.vector.tensor_copy(dst, src)

                return [mk_mm(kt) for kt in range(KT8)] + [copy]

            for ot in range(NP):
                items.extend(qk_group(ot, wq_t, QT, f"pq{st}{ot}"))
            for ot in range(NP):
                items.extend(qk_group(ot, wk_t, KTt, f"pk{st}{ot}"))
            for sub in range(4):
                items.extend(v_group(sub))
            return items

        def attention_block(j, fill, ctxn):
            """Causal attention + softmax-denominator for q-block j.
            `fill` is a list of closures (projections / output projections)
            sprinkled into the PE stream to cover exp-wait stalls. Normalized
            context tiles are appended to `ctxn` eagerly (the normalize chain
            runs on DVE+GpSimd only, so it never stalls the PE)."""
            fill = list(fill)
            n_triples = NP * 4 * (j + 1)
            per_triple = -(-len(fill) // n_triples) if fill else 0

            def emit_fill(n):
                for _ in range(n):
                    if not fill:
                        return
                    if fill[0]() is False:
                        return  # head item's inputs not produced yet
                    fill.pop(0)

            ni = 4 * (j + 1)

            def scores(pair, i):
                p = i - 4 * j
                lo = 128 * p if p > 0 else 0
                sc = sc_ps.tile([128, 1024], F32, tag="sc",
                                name=f"sc{j}{pair}{i}")
                qa = QT[pair][0:64, j * 512 + lo:(j + 1) * 512]
                qb = QT[pair][64:128, j * 512 + lo:(j + 1) * 512]
                ka = KTt[pair][0:64, i * 128:(i + 1) * 128]
                kb = KTt[pair][64:128, i * 128:(i + 1) * 128]
                nc.tensor.matmul(
                    sc[:, lo:512], ka, qa,
                    start=True, stop=True, tile_position=(0, 0),
                )
                nc.tensor.matmul(
                    sc[:, 512 + lo:1024], kb, qb,
                    start=True, stop=True, tile_position=(64, 0),
                )
                return sc

            def emit_exp(sc, i, pair):
                p = i - 4 * j
                et = exp_pool.tile([128, 1024], BF16, tag="exp",
                                   name=f"et{j}{pair}{i}")
                if p >= 2:
                    lo = 128 * p
                    nc.scalar.activation(
                        et[:, lo:512], sc[:, lo:512], AF.Exp)
                    nc.scalar.activation(
                        et[:, 512 + lo:1024], sc[:, 512 + lo:1024], AF.Exp)
                else:
                    nc.scalar.activation(et[:], sc[:], AF.Exp)
                if p >= 0:
                    # diagonal block: zero the future positions within the
                    # 128-wide triangle at [lo, lo+128): keep iff qq' >= kk
                    lo = 128 * p
                    ap = et[:].rearrange(
                        "p (h q) -> p h q", h=2)[:, :, lo:lo + 128]
                    nc.gpsimd.affine_select(
                        out=ap, in_=ap,
                        pattern=[[0, 2], [1, 128]],
                        compare_op=ALU.is_ge,
                        fill=0.0,
                        base=0,
                        channel_multiplier=-1,
                    )
                return et

            def emit_ctx(ctx_a, ctx_b, et, i):
                p = i - 4 * j
                lo = 128 * p if p > 0 else 0
                first, last = (i == 0), (i == ni - 1)
                va = V[i][:, pair * 2 * VW: pair * 2 * VW + VW]
                vb = V[i][:, pair * 2 * VW + VW: pair * 2 * VW + 2 * VW]
                nc.tensor.matmul(
                    ctx_a[:, lo:512], va, et[:, lo:512],
                    start=first, stop=last, skip_group_check=True,
                )
                nc.tensor.matmul(
                    ctx_b[:, lo:512], vb, et[:, 512 + lo:1024],
                    start=first, stop=last, skip_group_check=True,
                )

            def norm_part1(ctx_a, ctx_b, pair):
                """Drain ctx psum; fast-reciprocal the l rows (partition-
                aligned, read straight from PSUM). All DVE — no PE stall."""
                with nc.allow_low_precision("bf16/fp32r attention pipeline"):
                    par = (j * NP + pair) % 2 * 2
                    ra, rb = rec_t[par], rec_t[par + 1]
                    rabf, rbbf = rec_bf[par], rec_bf[par + 1]
                    # 1/l = exp(-ln(l)) on ScalarE (table set has both fns);
                    # avoids the slow DVE reciprocal entirely.
                    nc.scalar.activation(
                        ra[64:65, :], ctx_a[64:65, :], AF.Ln)
                    nc.scalar.activation(
                        rabf[64:65, :], ra[64:65, :], AF.Exp, scale=-1.0)
                    nc.scalar.activation(
                        rb[64:65, :], ctx_b[64:65, :], AF.Ln)
                    nc.scalar.activation(
                        rbbf[64:65, :], rb[64:65, :], AF.Exp, scale=-1.0)
                    cra = ctxraw_pool.tile([VW, 512], BF16, tag="cr",
                                           name=f"cra{j}{pair}")
                    nc.vector.tensor_copy(cra[:], ctx_a[:])
                    crb = ctxraw_pool.tile([VW, 512], BF16, tag="cr",
                                           name=f"crb{j}{pair}")
                    nc.vector.tensor_copy(crb[:], ctx_b[:])
                return rabf, rbbf, cra, crb

            def norm_part2(ra, rb, cra, crb, pair):
                """Broadcast 1/l onto rows 0-63 with a tiny matmul (same PE
                tiling mode as the fills), multiply, and DMA head b's rows
                across to partitions 64-127."""
                with nc.allow_low_precision("bf16/fp32r attention pipeline"):
                    cn = ctxn_pool.tile([128, 512], BF16, tag="cn",
                                        name=f"cn{j}{pair}")
                    bps_a = mm_ps.tile([128, 512], F32, tag="mm",
                                       name=f"bpa{j}{pair}")
                    nc.tensor.matmul(bps_a[:], bcA[:], ra[:],
                                     start=True, stop=True)
                    nc.vector.tensor_mul(cn[0:64, :], cra[0:64, :],
                                         bps_a[0:64, :])
                    bps_b = mm_ps.tile([128, 512], F32, tag="mm",
                                       name=f"bpb{j}{pair}")
                    nc.tensor.matmul(bps_b[:], bcA[:], rb[:],
                                     start=True, stop=True)
                    cnb = cnb_pool.tile([64, 512], BF16, tag="cnb",
                                        name=f"cnb{j}{pair}")
                    nc.vector.tensor_mul(cnb[:], crb[0:64, :],
                                         bps_b[0:64, :])
                    nc.gpsimd.dma_start(cn[64:128, :], cnb[:])
                ctxn.append(cn)

            pending_norm = None
            for pair in range(NP):
                ctx_a = ctx_ps.tile([VW, 512], F32, tag="ctx",
                                    name=f"ctxa{j}{pair}")
                ctx_b = ctx_ps.tile([VW, 512], F32, tag="ctx",
                                    name=f"ctxb{j}{pair}")
                sc = scores(pair, 0)
                pending = None
                for i in range(ni):
                    et = emit_exp(sc, i, pair)
                    if pending is not None:
                        emit_ctx(ctx_a, ctx_b, *pending)
                        emit_fill(per_triple)
                    if pending_norm is not None:
                        # by now the fast reciprocal is long done, so the
                        # broadcast matmuls dispatch without stalling the PE
                        norm_part2(*pending_norm)
                        pending_norm = None
                    if i + 1 < ni:
                        sc = scores(pair, i + 1)
                    pending = (et, i)
                emit_ctx(ctx_a, ctx_b, *pending)
                emit_fill(per_triple)
                pending_norm = norm_part1(ctx_a, ctx_b, pair) + (pair,)
            norm_part2(*pending_norm)

            # drain any remaining fill (all inputs exist by block end)
            while fill:
                assert fill[0]() is not False
                fill.pop(0)

        def outproj_items(j, ctxn):
            """Output projection for q-tile j as fine-grain fill items."""
            items = []

            def group(s4, oh, holder):
                def mk_mm(pair):
                    def go():
                        if len(ctxn) <= pair:
                            return False  # cn not normalized yet
                        if "ps" not in holder:
                            holder["ps"] = mm_ps.tile(
                                [128, 512], F32, tag="mm", name=f"yp{j}{s4}{oh}")
                        nc.tensor.matmul(
                            holder["ps"][:],
                            ctxn[pair][:, s4 * 128:(s4 + 1) * 128],
                            wo_t[pair][:, oh * 512:(oh + 1) * 512],
                            start=(pair == 0),
                            stop=(pair == NP - 1),
                        )
                    return go

                def copy():
                    nc.vector.tensor_copy(
                        holder["yb"][:, oh * 512:(oh + 1) * 512], holder["ps"][:])
                    del holder["ps"]

                return [mk_mm(p) for p in range(NP)] + [copy]

            for s4 in range(4):
                srow = j * 4 + s4
                holder = {}

                def alloc_yb(holder=holder, s4=s4):
                    holder["yb"] = ybuf_pool.tile(
                        [128, D], BF16, tag="yb", name=f"yb{j}{s4}")

                items.append(alloc_yb)
                for oh in range(2):
                    items.extend(group(s4, oh, holder))

                def dma_out(holder=holder, srow=srow):
                    nc.sync.dma_start(
                        yout[srow * 128:(srow + 1) * 128, :], holder["yb"][:])

                items.append(dma_out)
            return items

        # chunk 0 projections run alone (Q first, matching weight-DMA
        # arrival order); attention block j then carries chunk j+1's
        # projections and block j-1's output projection as PE filler for its
        # exp-wait stalls; block 3 additionally self-fills with its own
        # output projection (enabled by the eager, PE-free normalize).
        items0 = proj_items(0, xts0)
        for item in items0[:4 * (KT8 + 1)]:   # Q groups
            item()
        for kt in range(KT8):
            t = wk_pool.tile([128, HG], BF16, tag=f"w{kt}", name=f"wk{kt}")
            nc.sync.dma_start(t[:], wkT[kt * 128:(kt + 1) * 128, :])
            wk_t.append(t)
        for item in items0[4 * (KT8 + 1):8 * (KT8 + 1)]:   # K groups
            item()
        for kt in range(KT8):
            t = wv_pool.tile([128, HG], BF16, tag=f"w{kt}", name=f"wv{kt}")
            nc.sync.dma_start(t[:], wvT[kt * 128:(kt + 1) * 128, :])
            wv_t.append(t)
        for item in items0[8 * (KT8 + 1):]:   # V groups
            item()
        for c in range(NP):
            t = wo_pool.tile([128, D], BF16, tag=f"wo{c}")
            nc.sync.dma_start(t[:], woT[c * 128:(c + 1) * 128, :])
            wo_t.append(t)

        prev_out = []
        for j in range(4):
            fill = list(prev_out)
            if j + 1 < 4:
                xts = emit_xt_dmas(j + 1)
                fill = proj_items(j + 1, xts) + fill
            ctxn = []
            if j == 3:
                prev_out = outproj_items(j, ctxn)
                fill = fill + prev_out
                prev_out = []
            attention_block(j, fill, ctxn)
            if j < 3:
                prev_out = outproj_items(j, ctxn)
        for item in prev_out:
            item()

    _enforce_wait_limits(nc.m)
    return nc


_NC = None


def _get_nc():
    global _NC
    if _NC is None:
        _NC = build_nc()
    return _NC


def run(x, Wq, Wk, Wv, Wo, trace=False, trace_kwargs=None):
    """Returns (y, BassKernelResults)."""
    x = np.asarray(x, np.float32)
    scale = 1.0 / np.sqrt(DK)
    in_maps = []
    for core in range(N_CORES):
        b, g = core // 2, core % 2
        cols = slice(g * HG, (g + 1) * HG)
        bf = ml_dtypes.bfloat16
        in_maps.append({
            "xT": np.ascontiguousarray(x[b].T).astype(bf),
            "wqT": np.ascontiguousarray(
                np.asarray(Wq, np.float32).T[:, cols] * scale).astype(bf),
            "wkT": np.ascontiguousarray(
                np.asarray(Wk, np.float32).T[:, cols]).astype(bf),
            "wvT": np.ascontiguousarray(
                np.asarray(Wv, np.float32).T[:, cols]).astype(bf),
            "woT": np.ascontiguousarray(
                np.asarray(Wo, np.float32).T[cols, :]).astype(bf),
        })
    kw = dict(trace_kwargs or {})
    res = run_bass_kernel_spmd(
        _get_nc(), in_maps, list(range(N_CORES)), trace=trace, **kw
    )
    y = np.empty((B, S, D), np.float32)
    for b in range(B):
        y[b] = (res.results[2 * b]["y"].astype(np.float32)
                + res.results[2 * b + 1]["y"].astype(np.float32))
    return y, res


def kernel(x, Wq, Wk, Wv, Wo):
    y, _ = run(x, Wq, Wk, Wv, Wo)
    return y


# revision 28
# speedup vs baseline: 1.1627x; 1.0050x over previous
"""Multi-head causal self-attention on 8 Trainium2 NeuronCores.

Problem: x [4, 2048, 1024], Wq/Wk/Wv/Wo [1024, 1024] (applied as x @ W.T),
16 heads, dk=64, causal softmax, output [4, 2048, 1024], all fp32.

Sharding: 8 cores = 4 batches x 2 head-groups (8 heads each).
Each core computes QKV projections for its 8 heads, streaming causal
attention, and a partial output projection (Wo row-split). The host adds
the two partial outputs per batch element.

Per-core layouts (chosen so NO on-device transposes are needed):
  xT  [1024, 2048]  = x[b].T          (host-transposed)
  wqT [1024, 512]   = (Wq/8).T cols for this head group (1/sqrt(dk) folded)
  wkT [1024, 512], wvT [1024, 512]
  woT [512, 1024]   = Wo[:, cols].T
  QT/KT on chip as [feat, seq] (head pairs stacked on partitions),
  V as [seq, 65*8] bf16 with a ones column appended per head, so the
  AV matmul (stationary [V | 1], 65 outputs) yields both the context
  rows AND the softmax denominator row in one pass, in the same PE
  tiling mode (128x128) as the projection fills. Scores tiles
  [k=128, q=512] per head pair are exp'ed on ScalarE into bf16; the
  causal mask is applied with affine_select on the idle GpSimd engine;
  1/l is broadcast across partitions with gpsimd partition_broadcast.

Causal-wedge (diagonal) tiles are narrowed to their valid q-range, and
projection s-chunks / output-projection blocks are interleaved in
program order so TensorE, ScalarE and GpSimd work concurrently.
"""

import ml_dtypes
import numpy as np

import concourse.bass as bass
import concourse.mybir as mybir
import concourse.tile as tile
from concourse.bass_utils import run_bass_kernel_spmd
from concourse.vector_clock import ScopedClock

F32 = mybir.dt.float32
F32R = mybir.dt.float32r
BF16 = mybir.dt.bfloat16
AF = mybir.ActivationFunctionType
ALU = mybir.AluOpType

B, S, D = 4, 2048, 1024
H = 16
DK = 64
N_CORES = 8
HG = 512          # head-group width (8 heads x 64)


# ---------------------------------------------------------------------------
# This walrus accepts at most 1 sem wait per instruction (2 for
# EventSemaphore). Tile emits more in two places; both are fixed up here by
# moving excess waits onto preceding instructions on the same engine.
# ---------------------------------------------------------------------------
def _split_drain_and_barrier(self, tick_clock, wait_clock):
    nc = self.nc
    probe = nc.sync.nop(nofuse=True, hint="tile_drain_waits")
    wait_clock.add_sem_waits(
        probe.ins, ScopedClock({None: tick_clock.global_clock})
    )
    si = probe.ins.sync_info
    waits = list(si.on_wait) if si is not None else []
    if len(waits) > 1:
        probe.ins.sync_info = mybir.SyncInfo(on_wait=[waits[0]], on_update=[])
        for w in waits[1:]:
            n = nc.sync.nop(nofuse=True, hint="tile_drain_waits")
            n.ins.sync_info = mybir.SyncInfo(on_wait=[w], on_update=[])
    nc.sync.drain()
    nc.all_engine_barrier()
    popped = nc._tile_sem_poison_stack.pop()
    assert popped is self._sem_poison
    nc.clear_and_free_semaphores(list(self.sems.allocated().values()))
    nc.all_engine_barrier()


tile.TileContext._drain_and_barrier = _split_drain_and_barrier

_wsplit_counter = [0]


def _enforce_wait_limits(m):
    for fn in m.functions:
        for bb in fn.blocks:
            out = []
            changed = False
            for inst in bb.instructions:
                si = inst.sync_info
                cap = 2 if isinstance(inst, mybir.InstEventSemaphore) else 1
                if si is not None and len(si.on_wait) > cap:
                    waits = list(si.on_wait)
                    keep, extra = waits[:cap], waits[cap:]
                    for i in range(0, len(extra), 2):
                        _wsplit_counter[0] += 1
                        out.append(mybir.InstEventSemaphore(
                            name=f"I-wsplit-{_wsplit_counter[0]}",
                            engine=inst.engine,
                            ins=[], outs=[],
                            sync_info=mybir.SyncInfo(
                                on_wait=extra[i:i + 2], on_update=[]),
                        ))
                    inst.sync_info = mybir.SyncInfo(
                        on_wait=keep, on_update=list(si.on_update))
                    changed = True
                out.append(inst)
            if changed:
                bb.instructions = out


def build_nc():
    nc = bass.Bass()

    xT = nc.declare_dram_parameter("xT", [D, S], BF16, isOutput=False)
    wqT = nc.declare_dram_parameter("wqT", [D, HG], BF16, isOutput=False)
    wkT = nc.declare_dram_parameter("wkT", [D, HG], BF16, isOutput=False)
    wvT = nc.declare_dram_parameter("wvT", [D, HG], BF16, isOutput=False)
    woT = nc.declare_dram_parameter("woT", [HG, D], BF16, isOutput=False)
    bc65 = nc.declare_dram_parameter("bc65", [65, 128], F32R, isOutput=False)
    zer64 = nc.declare_dram_parameter("zer64", [64, 512], F32R,
                                      isOutput=False)
    yout = nc.declare_dram_parameter("y", [S, D], BF16, isOutput=True)

    KT8 = D // 128   # contraction tiles for the projections
    NP = 4           # head pairs per core
    NS = S // 128    # seq tiles of 128
    VW = 65          # per-head V width in the augmented layout ([V | 1])

    from contextlib import ExitStack

    with tile.TileContext(nc) as tc, ExitStack() as ctx:
        ep = ctx.enter_context
        consts = ep(tc.tile_pool(name="consts", bufs=1))
        qt_pool = ep(tc.tile_pool(name="qt", bufs=1))
        kt_pool = ep(tc.tile_pool(name="kt", bufs=1))
        v_pool = ep(tc.tile_pool(name="v", bufs=1))
        wo_pool = ep(tc.tile_pool(name="wo", bufs=1))
        wq_pool = ep(tc.tile_pool(name="wq", bufs=1))
        wk_pool = ep(tc.tile_pool(name="wk", bufs=1))
        wv_pool = ep(tc.tile_pool(name="wv", bufs=1))
        xt_pool = ep(tc.tile_pool(name="xt", bufs=2))
        exp_pool = ep(tc.tile_pool(name="exp", bufs=3))
        ctxn_pool = ep(tc.tile_pool(name="ctxn", bufs=12))
        cnb_pool = ep(tc.tile_pool(name="cnb", bufs=2))
        ctxraw_pool = ep(tc.tile_pool(name="ctxraw", bufs=4))
        lrow_pool = ep(tc.tile_pool(name="lrow", bufs=2))
        lpack_pool = ep(tc.tile_pool(name="lpack", bufs=4))
        dram_pool = ep(tc.tile_pool(name="ldram", bufs=2, space="DRAM"))
        ybuf_pool = ep(tc.tile_pool(name="ybuf", bufs=2))
        mm_ps = ep(tc.tile_pool(name="mm_ps", bufs=2, space="PSUM"))
        sc_ps = ep(tc.tile_pool(name="sc_ps", bufs=2, space="PSUM"))
        ctx_ps = ep(tc.tile_pool(name="ctx_ps", bufs=2, space="PSUM"))

        QT = [qt_pool.tile([128, S], BF16, tag=f"qt{p}", name=f"QT{p}")
              for p in range(NP)]
        KTt = [kt_pool.tile([128, S], BF16, tag=f"kt{p}", name=f"KTt{p}")
               for p in range(NP)]
        # V_aug: per seq-tile, [128, 8*65]: per head 64 V columns + a ones
        # column, so the AV stationary [V | 1] produces ctx rows 0-63 and
        # the softmax denominator at row 64.
        V = [v_pool.tile([128, 8 * VW], BF16, tag=f"v{s}", name=f"V{s}")
             for s in range(NS)]

        # ---- weight/x DMAs: batched into a few large transfers so the
        # issuing queues don't serialize startup. The first Q matmuls only
        # wait for the first wq/x slices.
        # x chunk: one [128, 8*512] tile, kt-major (cols kt*512..kt*512+511).
        xT_v = xT.rearrange("(a p) s -> p a s", p=128)     # [128, 8, 2048]
        wq_v = wqT.rearrange("(a p) h -> p a h", p=128)    # [128, 8, 512]
        wk_v = wkT.rearrange("(a p) h -> p a h", p=128)
        wv_v = wvT.rearrange("(a p) h -> p a h", p=128)
        wo_v = woT.rearrange("(a p) d -> p a d", p=128)    # [128, 4, 1024]

        def emit_xt_dmas(st, split=1):
            t = xt_pool.tile([128, KT8 * 512], BF16, tag="xt", name=f"xt{st}")
            tv = t[:].rearrange("p (a s) -> p a s", s=512)
            step = KT8 // split
            for i in range(split):
                nc.gpsimd.dma_start(
                    tv[:, i * step:(i + 1) * step, :],
                    xT_v[:, i * step:(i + 1) * step,
                         st * 512:(st + 1) * 512])
            return t

        xts0 = emit_xt_dmas(0, split=4)
        wq_t = wq_pool.tile([128, KT8 * 512], BF16, tag="wq", name="wq")
        wqv = wq_t[:].rearrange("p (a h) -> p a h", h=512)
        nc.sync.dma_start(wqv[:, 0:4, :], wq_v[:, 0:4, :])
        nc.sync.dma_start(wqv[:, 4:8, :], wq_v[:, 4:8, :])
        wk_t = wk_pool.tile([128, KT8 * 512], BF16, tag="wk", name="wk")
        nc.sync.dma_start(
            wk_t[:].rearrange("p (a h) -> p a h", h=512), wk_v[:])
        wv_t = wv_pool.tile([128, KT8 * 512], BF16, tag="wv", name="wv")
        nc.sync.dma_start(
            wv_t[:].rearrange("p (a h) -> p a h", h=512), wv_v[:])
        wo_t = wo_pool.tile([128, NP * D], BF16, tag="wo", name="wo")
        nc.sync.dma_start(
            wo_t[:].rearrange("p (a d) -> p a d", d=D), wo_v[:])
        # broadcast constant: bcA.T @ m replicates m's row 64 onto rows 0-63
        bcA = consts.tile([VW, 128], F32R, tag="bcA", name="bcA")
        # static scratch rows for 1/l (row 64 live, rows 0-63 stay zero)
        rec_t = [consts.tile([VW, 512], F32R, tag=f"rec{i}", name=f"rec{i}")
                 for i in range(4)]

        def proj_items(st, xts):
            """QKV projection work for chunk st as a flat list of closures,
            one instruction each, so they can be sprinkled between attention
            stages at fine grain."""
            items = []

            def qk_group(ot, w_t, dst, name):
                holder = {}

                def mk_mm(kt):
                    def go():
                        if "ps" not in holder:
                            holder["ps"] = mm_ps.tile(
                                [128, 512], F32, tag="mm", name=name)
                        nc.tensor.matmul(
                            holder["ps"][:],
                            w_t[:, kt * 512 + ot * 128:
                                kt * 512 + (ot + 1) * 128],
                            xts[:, kt * 512:(kt + 1) * 512],
                            start=(kt == 0),
                            stop=(kt == KT8 - 1),
                        )
                    return go

                def copy():
                    nc.vector.tensor_copy(
                        dst[ot][:, st * 512:(st + 1) * 512], holder["ps"][:])

                return [mk_mm(kt) for kt in range(KT8)] + [copy]

            def v_group(sub):
                holder = {}

                def mk_mm(kt):
                    def go():
                        if "ps" not in holder:
                            holder["ps"] = mm_ps.tile(
                                [128, 512], F32, tag="mm", name=f"pv{st}{sub}")
                        nc.tensor.matmul(
                            holder["ps"][:],
                            xts[:, kt * 512 + sub * 128:
                                kt * 512 + (sub + 1) * 128],
                            wv_t[:, kt * 512:(kt + 1) * 512],
                            start=(kt == 0),
                            stop=(kt == KT8 - 1),
                        )
                    return go

                def copy():
                    dst = V[st * 4 + sub][:].rearrange(
                        "p (h c) -> p h c", c=VW)[:, :, 0:64]
                    src = holder["ps"][:].rearrange("p (h c) -> p h c", c=64)
                    nc.vector.tensor_copy(dst, src)

                return [mk_mm(kt) for kt in range(KT8)] + [copy]

            for ot in range(NP):
                items.extend(qk_group(ot, wq_t, QT, f"pq{st}{ot}"))
            for ot in range(NP):
                items.extend(qk_group(ot, wk_t, KTt, f"pk{st}{ot}"))
            for sub in range(4):
                items.extend(v_group(sub))
            return items

        def norm_part1(ctx_a, ctx_b, j, pair, ctxn, fast=False):
            """Drain ctx psum (incl. the l rows at 64) and compute 1/l.
            Default path: repack both heads' l through DRAM onto all 128
            partitions so the DVE reciprocal costs ~200ns; the chain is all
            DVE+DMA and is consumed a full pair later (norm_part2), so the
            PE never waits on it. `fast=True` computes 1/l as exp(-ln(l)) on
            ScalarE instead — a short-latency chain for the final pair."""
            par = (j * NP + pair) % 2 * 2
            ra, rb = rec_t[par], rec_t[par + 1]
            with nc.allow_low_precision("bf16/fp32r attention pipeline"):
                cra = ctxraw_pool.tile([VW, 512], BF16, tag="cr",
                                       name=f"cra{j}{pair}")
                nc.vector.tensor_copy(cra[:], ctx_a[:])
                crb = ctxraw_pool.tile([VW, 512], BF16, tag="cr",
                                       name=f"crb{j}{pair}")
                nc.vector.tensor_copy(crb[:], ctx_b[:])
                if fast:
                    lrow = lrow_pool.tile([VW, 1024], F32, tag="lr",
                                          name=f"lr{j}{pair}")
                    nc.scalar.activation(
                        lrow[64:65, 0:512], ctx_a[64:65, :], AF.Ln)
                    nc.scalar.activation(
                        ra[64:65, :], lrow[64:65, 0:512],
                        AF.Exp, scale=-1.0)
                    nc.scalar.activation(
                        lrow[64:65, 512:1024], ctx_b[64:65, :], AF.Ln)
                    nc.scalar.activation(
                        rb[64:65, :], lrow[64:65, 512:1024],
                        AF.Exp, scale=-1.0)
                else:
                    lrow = lrow_pool.tile([VW, 1024], F32, tag="lr",
                                          name=f"lr{j}{pair}")
                    nc.vector.tensor_copy(
                        lrow[64:65, 0:512], ctx_a[64:65, :])
                    nc.vector.tensor_copy(
                        lrow[64:65, 512:1024], ctx_b[64:65, :])
                    ls = dram_pool.tile([1, 1024], F32, tag="ls",
                                        name=f"ls{j}{pair}")
                    nc.sync.dma_start(ls[:], lrow[64:65, :])
                    lpack = lpack_pool.tile([128, 8], F32, tag="lp",
                                            name=f"lp{j}{pair}")
                    nc.sync.dma_start(
                        lpack[:], ls[0].rearrange("(p f) -> p f", p=128))
                    rpk = lpack_pool.tile([128, 8], F32R, tag="rp",
                                          name=f"rp{j}{pair}")
                    nc.vector.reciprocal(rpk[:], lpack[:])
                    rs = dram_pool.tile([1, 1024], F32R, tag="rs",
                                        name=f"rs{j}{pair}")
                    nc.sync.dma_start(
                        rs[0].rearrange("(p f) -> p f", p=128), rpk[:])
                    nc.sync.dma_start(ra[64:65, :], rs[0:1, 0:512])
                    nc.sync.dma_start(rb[64:65, :], rs[0:1, 512:1024])
            return (ra, rb, cra, crb, j, pair, ctxn)

        def norm_part2(ra, rb, cra, crb, j, pair, ctxn):
            """Broadcast 1/l onto rows 0-63 with a tiny matmul (same PE
            tiling mode as the fills), multiply, and DMA head b's rows
            across to partitions 64-127."""
            with nc.allow_low_precision("bf16/fp32r attention pipeline"):
                cn = ctxn_pool.tile([128, 512], BF16, tag="cn",
                                    name=f"cn{j}{pair}")
                bps_a = mm_ps.tile([128, 512], F32, tag="mm",
                                   name=f"bpa{j}{pair}")
                nc.tensor.matmul(bps_a[:], bcA[:], ra[:],
                                 start=True, stop=True)
                nc.vector.tensor_mul(cn[0:64, :], cra[0:64, :],
                                     bps_a[0:64, :])
                bps_b = mm_ps.tile([128, 512], F32, tag="mm",
                                   name=f"bpb{j}{pair}")
                nc.tensor.matmul(bps_b[:], bcA[:], rb[:],
                                 start=True, stop=True)
                cnb = cnb_pool.tile([64, 512], BF16, tag="cnb",
                                    name=f"cnb{j}{pair}")
                nc.vector.tensor_mul(cnb[:], crb[0:64, :],
                                     bps_b[0:64, :])
                nc.gpsimd.dma_start(cn[64:128, :], cnb[:])
            ctxn.append(cn)

        def attention_block(j, fill, ctxn, carried):
            """Causal attention + softmax-denominator for q-block j.
            `fill` is a list of closures (projections / output projections)
            sprinkled into the PE stream to cover exp-wait stalls. `carried`
            is the previous pair's unfinished normalize (possibly from the
            previous block); the one left over here is returned."""
            fill = list(fill)
            n_triples = NP * 4 * (j + 1)
            per_triple = -(-len(fill) // n_triples) if fill else 0

            def emit_fill(n):
                for _ in range(n):
                    if not fill:
                        return
                    if fill[0]() is False:
                        return  # head item's inputs not produced yet
                    fill.pop(0)

            ni = 4 * (j + 1)

            def scores(pair, i):
                p = i - 4 * j
                lo = 128 * p if p > 0 else 0
                sc = sc_ps.tile([128, 1024], F32, tag="sc",
                                name=f"sc{j}{pair}{i}")
                qa = QT[pair][0:64, j * 512 + lo:(j + 1) * 512]
                qb = QT[pair][64:128, j * 512 + lo:(j + 1) * 512]
                ka = KTt[pair][0:64, i * 128:(i + 1) * 128]
                kb = KTt[pair][64:128, i * 128:(i + 1) * 128]
                nc.tensor.matmul(
                    sc[:, lo:512], ka, qa,
                    start=True, stop=True, tile_position=(0, 0),
                )
                nc.tensor.matmul(
                    sc[:, 512 + lo:1024], kb, qb,
                    start=True, stop=True, tile_position=(64, 0),
                )
                return sc

            def emit_exp(sc, i, pair):
                p = i - 4 * j
                et = exp_pool.tile([128, 1024], BF16, tag="exp",
                                   name=f"et{j}{pair}{i}")
                if p >= 2:
                    lo = 128 * p
                    nc.scalar.activation(
                        et[:, lo:512], sc[:, lo:512], AF.Exp)
                    nc.scalar.activation(
                        et[:, 512 + lo:1024], sc[:, 512 + lo:1024], AF.Exp)
                else:
                    nc.scalar.activation(et[:], sc[:], AF.Exp)
                if p >= 0:
                    # diagonal block: zero the future positions within the
                    # 128-wide triangle at [lo, lo+128): keep iff qq' >= kk
                    lo = 128 * p
                    ap = et[:].rearrange(
                        "p (h q) -> p h q", h=2)[:, :, lo:lo + 128]
                    nc.gpsimd.affine_select(
                        out=ap, in_=ap,
                        pattern=[[0, 2], [1, 128]],
                        compare_op=ALU.is_ge,
                        fill=0.0,
                        base=0,
                        channel_multiplier=-1,
                    )
                return et

            def emit_ctx(ctx_a, ctx_b, et, i):
                p = i - 4 * j
                lo = 128 * p if p > 0 else 0
                first, last = (i == 0), (i == ni - 1)
                va = V[i][:, pair * 2 * VW: pair * 2 * VW + VW]
                vb = V[i][:, pair * 2 * VW + VW: pair * 2 * VW + 2 * VW]
                nc.tensor.matmul(
                    ctx_a[:, lo:512], va, et[:, lo:512],
                    start=first, stop=last, skip_group_check=True,
                )
                nc.tensor.matmul(
                    ctx_b[:, lo:512], vb, et[:, 512 + lo:1024],
                    start=first, stop=last, skip_group_check=True,
                )

            for pair in range(NP):
                ctx_a = ctx_ps.tile([VW, 512], F32, tag="ctx",
                                    name=f"ctxa{j}{pair}")
                ctx_b = ctx_ps.tile([VW, 512], F32, tag="ctx",
                                    name=f"ctxb{j}{pair}")
                sc = scores(pair, 0)
                pending = None
                for i in range(ni):
                    et = emit_exp(sc, i, pair)
                    if pending is not None:
                        emit_ctx(ctx_a, ctx_b, *pending)
                        emit_fill(per_triple)
                    if carried is not None and i >= 1:
                        # a full pair has passed — the reciprocal repack
                        # chain is done, so the broadcast matmuls dispatch
                        # without stalling the PE
                        norm_part2(*carried)
                        carried = None
                    if i + 1 < ni:
                        sc = scores(pair, i + 1)
                    pending = (et, i)
                emit_ctx(ctx_a, ctx_b, *pending)
                emit_fill(per_triple)
                fast = (j == 3 and pair == NP - 1)
                carried = norm_part1(ctx_a, ctx_b, j, pair, ctxn, fast=fast)
            if j == 3:
                norm_part2(*carried)
                carried = None

            # drain any remaining fill (all inputs exist by block end)
            while fill:
                assert fill[0]() is not False
                fill.pop(0)
            return carried

        def outproj_items(j, ctxn):
            """Output projection for q-tile j as fine-grain fill items."""
            items = []

            def group(s4, oh, holder):
                def mk_mm(pair):
                    def go():
                        if len(ctxn) <= pair:
                            return False  # cn not normalized yet
                        if "ps" not in holder:
                            holder["ps"] = mm_ps.tile(
                                [128, 512], F32, tag="mm", name=f"yp{j}{s4}{oh}")
                        nc.tensor.matmul(
                            holder["ps"][:],
                            ctxn[pair][:, s4 * 128:(s4 + 1) * 128],
                            wo_t[:, pair * D + oh * 512:
                                 pair * D + (oh + 1) * 512],
                            start=(pair == 0),
                            stop=(pair == NP - 1),
                        )
                    return go

                def copy():
                    nc.vector.tensor_copy(
                        holder["yb"][:, oh * 512:(oh + 1) * 512], holder["ps"][:])
                    del holder["ps"]

                return [mk_mm(p) for p in range(NP)] + [copy]

            for s4 in range(4):
                srow = j * 4 + s4
                holder = {}

                def alloc_yb(holder=holder, s4=s4):
                    holder["yb"] = ybuf_pool.tile(
                        [128, D], BF16, tag="yb", name=f"yb{j}{s4}")

                items.append(alloc_yb)
                for oh in range(2):
                    items.extend(group(s4, oh, holder))

                def dma_out(holder=holder, srow=srow):
                    nc.sync.dma_start(
                        yout[srow * 128:(srow + 1) * 128, :], holder["yb"][:])

                items.append(dma_out)
            return items

        # chunk 0 projections run alone (Q first, matching weight-DMA
        # arrival order); attention block j then carries chunk j+1's
        # projections and block j-1's output projection as PE filler for its
        # exp-wait stalls; block 3 additionally self-fills with its own
        # output projection (enabled by the eager, PE-free normalize).
        items0 = proj_items(0, xts0)
        for item in items0[:4 * (KT8 + 1)]:   # Q groups
            item()
        # constants + V ones-columns: emitted here so the loads don't
        # delay the startup DMAs
        nc.sync.dma_start(bcA[:], bc65[:])
        for t in rec_t:
            nc.sync.dma_start(t[0:64, :], zer64[:])
        for s in range(NS):
            ones_ap = V[s][:].rearrange("p (h c) -> p h c", c=VW)[:, :, 64:65]
            nc.gpsimd.memset(ones_ap, 1.0)
        for item in items0[4 * (KT8 + 1):8 * (KT8 + 1)]:   # K groups
            item()
        for item in items0[8 * (KT8 + 1):]:   # V groups
            item()

        prev_out = []
        carried = None
        for j in range(4):
            fill = list(prev_out)
            if j + 1 < 4:
                xts = emit_xt_dmas(j + 1)
                fill = proj_items(j + 1, xts) + fill
            ctxn = []
            if j == 3:
                prev_out = outproj_items(j, ctxn)
                fill = fill + prev_out
                prev_out = []
            carried = attention_block(j, fill, ctxn, carried)
            if j < 3:
                prev_out = outproj_items(j, ctxn)
        for item in prev_out:
            item()

    _enforce_wait_limits(nc.m)
    return nc


_NC = None


def _get_nc():
    global _NC
    if _NC is None:
        _NC = build_nc()
    return _NC


def run(x, Wq, Wk, Wv, Wo, trace=False, trace_kwargs=None):
    """Returns (y, BassKernelResults)."""
    x = np.asarray(x, np.float32)
    scale = 1.0 / np.sqrt(DK)
    bc = np.zeros((65, 128), np.float32)
    bc[64, 0:64] = 1.0
    zer = np.zeros((64, 512), np.float32)
    in_maps = []
    for core in range(N_CORES):
        b, g = core // 2, core % 2
        cols = slice(g * HG, (g + 1) * HG)
        bf = ml_dtypes.bfloat16
        in_maps.append({
            "xT": np.ascontiguousarray(x[b].T).astype(bf),
            "wqT": np.ascontiguousarray(
                np.asarray(Wq, np.float32).T[:, cols] * scale).astype(bf),
            "wkT": np.ascontiguousarray(
                np.asarray(Wk, np.float32).T[:, cols]).astype(bf),
            "wvT": np.ascontiguousarray(
                np.asarray(Wv, np.float32).T[:, cols]).astype(bf),
            "woT": np.ascontiguousarray(
                np.asarray(Wo, np.float32).T[cols, :]).astype(bf),
            "bc65": bc,
            "zer64": zer,
        })
    kw = dict(trace_kwargs or {})
    res = run_bass_kernel_spmd(
        _get_nc(), in_maps, list(range(N_CORES)), trace=trace, **kw
    )
    y = np.empty((B, S, D), np.float32)
    for b in range(B):
        y[b] = (res.results[2 * b]["y"].astype(np.float32)
                + res.results[2 * b + 1]["y"].astype(np.float32))
    return y, res


def kernel(x, Wq, Wk, Wv, Wo):
    y, _ = run(x, Wq, Wk, Wv, Wo)
    return y


# revision 30
# speedup vs baseline: 1.1929x; 1.0260x over previous
"""Multi-head causal self-attention on 8 Trainium2 NeuronCores.

Problem: x [4, 2048, 1024], Wq/Wk/Wv/Wo [1024, 1024] (applied as x @ W.T),
16 heads, dk=64, causal softmax, output [4, 2048, 1024], all fp32.

Sharding: 8 cores = 4 batches x 2 head-groups (8 heads each).
Each core computes QKV projections for its 8 heads, streaming causal
attention, and a partial output projection (Wo row-split). The host adds
the two partial outputs per batch element.

Per-core layouts (chosen so NO on-device transposes are needed):
  xT  [1024, 2048]  = x[b].T          (host-transposed)
  wqT [1024, 512]   = (Wq/8).T cols for this head group (1/sqrt(dk) folded)
  wkT [1024, 512], wvT [1024, 512]
  woT [512, 1024]   = Wo[:, cols].T
  QT/KT on chip as [feat, seq] (head pairs stacked on partitions),
  V as [seq, 65*8] bf16 with a ones column appended per head, so the
  AV matmul (stationary [V | 1], 65 outputs) yields both the context
  rows AND the softmax denominator row in one pass, in the same PE
  tiling mode (128x128) as the projection fills. Scores tiles
  [k=128, q=512] per head pair are exp'ed on ScalarE into bf16; the
  causal mask is applied with affine_select on the idle GpSimd engine;
  1/l is broadcast across partitions with gpsimd partition_broadcast.

Causal-wedge (diagonal) tiles are narrowed to their valid q-range, and
projection s-chunks / output-projection blocks are interleaved in
program order so TensorE, ScalarE and GpSimd work concurrently.
"""

import ml_dtypes
import numpy as np

import concourse.bass as bass
import concourse.mybir as mybir
import concourse.tile as tile
from concourse.bass_utils import run_bass_kernel_spmd
from concourse.vector_clock import ScopedClock

F32 = mybir.dt.float32
F32R = mybir.dt.float32r
BF16 = mybir.dt.bfloat16
AF = mybir.ActivationFunctionType
ALU = mybir.AluOpType

B, S, D = 4, 2048, 1024
H = 16
DK = 64
N_CORES = 8
HG = 512          # head-group width (8 heads x 64)


# ---------------------------------------------------------------------------
# This walrus accepts at most 1 sem wait per instruction (2 for
# EventSemaphore). Tile emits more in two places; both are fixed up here by
# moving excess waits onto preceding instructions on the same engine.
# ---------------------------------------------------------------------------
def _split_drain_and_barrier(self, tick_clock, wait_clock):
    nc = self.nc
    probe = nc.sync.nop(nofuse=True, hint="tile_drain_waits")
    wait_clock.add_sem_waits(
        probe.ins, ScopedClock({None: tick_clock.global_clock})
    )
    si = probe.ins.sync_info
    waits = list(si.on_wait) if si is not None else []
    if len(waits) > 1:
        probe.ins.sync_info = mybir.SyncInfo(on_wait=[waits[0]], on_update=[])
        for w in waits[1:]:
            n = nc.sync.nop(nofuse=True, hint="tile_drain_waits")
            n.ins.sync_info = mybir.SyncInfo(on_wait=[w], on_update=[])
    nc.sync.drain()
    nc.all_engine_barrier()
    popped = nc._tile_sem_poison_stack.pop()
    assert popped is self._sem_poison
    nc.clear_and_free_semaphores(list(self.sems.allocated().values()))
    nc.all_engine_barrier()


tile.TileContext._drain_and_barrier = _split_drain_and_barrier

_wsplit_counter = [0]


def _enforce_wait_limits(m):
    for fn in m.functions:
        for bb in fn.blocks:
            out = []
            changed = False
            for inst in bb.instructions:
                si = inst.sync_info
                cap = 2 if isinstance(inst, mybir.InstEventSemaphore) else 1
                if si is not None and len(si.on_wait) > cap:
                    waits = list(si.on_wait)
                    keep, extra = waits[:cap], waits[cap:]
                    for i in range(0, len(extra), 2):
                        _wsplit_counter[0] += 1
                        out.append(mybir.InstEventSemaphore(
                            name=f"I-wsplit-{_wsplit_counter[0]}",
                            engine=inst.engine,
                            ins=[], outs=[],
                            sync_info=mybir.SyncInfo(
                                on_wait=extra[i:i + 2], on_update=[]),
                        ))
                    inst.sync_info = mybir.SyncInfo(
                        on_wait=keep, on_update=list(si.on_update))
                    changed = True
                out.append(inst)
            if changed:
                bb.instructions = out


def build_nc():
    nc = bass.Bass()

    xT = nc.declare_dram_parameter("xT", [D, S], BF16, isOutput=False)
    wqT = nc.declare_dram_parameter("wqT", [D, HG], BF16, isOutput=False)
    wkT = nc.declare_dram_parameter("wkT", [D, HG], BF16, isOutput=False)
    wvT = nc.declare_dram_parameter("wvT", [D, HG], BF16, isOutput=False)
    woT = nc.declare_dram_parameter("woT", [HG, D], BF16, isOutput=False)
    bc65 = nc.declare_dram_parameter("bc65", [65, 128], F32R, isOutput=False)
    zer64 = nc.declare_dram_parameter("zer64", [64, 512], F32R,
                                      isOutput=False)
    yout = nc.declare_dram_parameter("y", [S, D], BF16, isOutput=True)

    KT8 = D // 128   # contraction tiles for the projections
    NP = 4           # head pairs per core
    NS = S // 128    # seq tiles of 128
    VW = 65          # per-head V width in the augmented layout ([V | 1])

    from contextlib import ExitStack

    with tile.TileContext(nc) as tc, ExitStack() as ctx:
        ep = ctx.enter_context
        consts = ep(tc.tile_pool(name="consts", bufs=1))
        qt_pool = ep(tc.tile_pool(name="qt", bufs=1))
        kt_pool = ep(tc.tile_pool(name="kt", bufs=1))
        v_pool = ep(tc.tile_pool(name="v", bufs=1))
        wo_pool = ep(tc.tile_pool(name="wo", bufs=1))
        wq_pool = ep(tc.tile_pool(name="wq", bufs=1))
        wk_pool = ep(tc.tile_pool(name="wk", bufs=1))
        wv_pool = ep(tc.tile_pool(name="wv", bufs=1))
        xt_pool = ep(tc.tile_pool(name="xt", bufs=2))
        exp_pool = ep(tc.tile_pool(name="exp", bufs=6))
        ctxn_pool = ep(tc.tile_pool(name="ctxn", bufs=12))
        cnb_pool = ep(tc.tile_pool(name="cnb", bufs=2))
        ctxraw_pool = ep(tc.tile_pool(name="ctxraw", bufs=4))
        lrow_pool = ep(tc.tile_pool(name="lrow", bufs=2))
        lpack_pool = ep(tc.tile_pool(name="lpack", bufs=4))
        dram_pool = ep(tc.tile_pool(name="ldram", bufs=2, space="DRAM"))
        ybuf_pool = ep(tc.tile_pool(name="ybuf", bufs=2))
        mm_ps = ep(tc.tile_pool(name="mm_ps", bufs=2, space="PSUM"))
        sc_ps = ep(tc.tile_pool(name="sc_ps", bufs=2, space="PSUM"))
        ctx_ps = ep(tc.tile_pool(name="ctx_ps", bufs=2, space="PSUM"))

        QT = [qt_pool.tile([128, S], BF16, tag=f"qt{p}", name=f"QT{p}")
              for p in range(NP)]
        KTt = [kt_pool.tile([128, S], BF16, tag=f"kt{p}", name=f"KTt{p}")
               for p in range(NP)]
        # V_aug: per seq-tile, [128, 8*65]: per head 64 V columns + a ones
        # column, so the AV stationary [V | 1] produces ctx rows 0-63 and
        # the softmax denominator at row 64.
        V = [v_pool.tile([128, 8 * VW], BF16, tag=f"v{s}", name=f"V{s}")
             for s in range(NS)]

        # ---- weight/x DMAs: batched into a few large transfers so the
        # issuing queues don't serialize startup. The first Q matmuls only
        # wait for the first wq/x slices.
        # x chunk: one [128, 8*512] tile, kt-major (cols kt*512..kt*512+511).
        xT_v = xT.rearrange("(a p) s -> p a s", p=128)     # [128, 8, 2048]
        wq_v = wqT.rearrange("(a p) h -> p a h", p=128)    # [128, 8, 512]
        wk_v = wkT.rearrange("(a p) h -> p a h", p=128)
        wv_v = wvT.rearrange("(a p) h -> p a h", p=128)
        wo_v = woT.rearrange("(a p) d -> p a d", p=128)    # [128, 4, 1024]

        def emit_xt_dmas(st, split=1):
            t = xt_pool.tile([128, KT8 * 512], BF16, tag="xt", name=f"xt{st}")
            tv = t[:].rearrange("p (a s) -> p a s", s=512)
            step = KT8 // split
            for i in range(split):
                nc.gpsimd.dma_start(
                    tv[:, i * step:(i + 1) * step, :],
                    xT_v[:, i * step:(i + 1) * step,
                         st * 512:(st + 1) * 512])
            return t

        xts0 = emit_xt_dmas(0, split=4)
        wq_t = wq_pool.tile([128, KT8 * 512], BF16, tag="wq", name="wq")
        wqv = wq_t[:].rearrange("p (a h) -> p a h", h=512)
        nc.sync.dma_start(wqv[:, 0:4, :], wq_v[:, 0:4, :])
        nc.sync.dma_start(wqv[:, 4:8, :], wq_v[:, 4:8, :])
        wk_t = wk_pool.tile([128, KT8 * 512], BF16, tag="wk", name="wk")
        nc.sync.dma_start(
            wk_t[:].rearrange("p (a h) -> p a h", h=512), wk_v[:])
        wv_t = wv_pool.tile([128, KT8 * 512], BF16, tag="wv", name="wv")
        nc.sync.dma_start(
            wv_t[:].rearrange("p (a h) -> p a h", h=512), wv_v[:])
        wo_t = wo_pool.tile([128, NP * D], BF16, tag="wo", name="wo")
        nc.sync.dma_start(
            wo_t[:].rearrange("p (a d) -> p a d", d=D), wo_v[:])
        # broadcast constant: bcA.T @ m replicates m's row 64 onto rows 0-63
        bcA = consts.tile([VW, 128], F32R, tag="bcA", name="bcA")
        # static scratch rows for 1/l (row 64 live, rows 0-63 stay zero)
        rec_t = [consts.tile([VW, 512], F32R, tag=f"rec{i}", name=f"rec{i}")
                 for i in range(4)]

        def proj_items(st, xts):
            """QKV projection work for chunk st as a flat list of closures,
            one instruction each, so they can be sprinkled between attention
            stages at fine grain."""
            items = []

            def qk_group(ot, w_t, dst, name):
                holder = {}

                def mk_mm(kt):
                    def go():
                        if "ps" not in holder:
                            holder["ps"] = mm_ps.tile(
                                [128, 512], F32, tag="mm", name=name)
                        nc.tensor.matmul(
                            holder["ps"][:],
                            w_t[:, kt * 512 + ot * 128:
                                kt * 512 + (ot + 1) * 128],
                            xts[:, kt * 512:(kt + 1) * 512],
                            start=(kt == 0),
                            stop=(kt == KT8 - 1),
                        )
                    return go

                def copy():
                    nc.vector.tensor_copy(
                        dst[ot][:, st * 512:(st + 1) * 512], holder["ps"][:])

                return [mk_mm(kt) for kt in range(KT8)] + [copy]

            def v_group(sub):
                holder = {}

                def mk_mm(kt):
                    def go():
                        if "ps" not in holder:
                            holder["ps"] = mm_ps.tile(
                                [128, 512], F32, tag="mm", name=f"pv{st}{sub}")
                        nc.tensor.matmul(
                            holder["ps"][:],
                            xts[:, kt * 512 + sub * 128:
                                kt * 512 + (sub + 1) * 128],
                            wv_t[:, kt * 512:(kt + 1) * 512],
                            start=(kt == 0),
                            stop=(kt == KT8 - 1),
                        )
                    return go

                def copy():
                    dst = V[st * 4 + sub][:].rearrange(
                        "p (h c) -> p h c", c=VW)[:, :, 0:64]
                    src = holder["ps"][:].rearrange("p (h c) -> p h c", c=64)
                    nc.vector.tensor_copy(dst, src)

                return [mk_mm(kt) for kt in range(KT8)] + [copy]

            for ot in range(NP):
                items.extend(qk_group(ot, wq_t, QT, f"pq{st}{ot}"))
            for ot in range(NP):
                items.extend(qk_group(ot, wk_t, KTt, f"pk{st}{ot}"))
            for sub in range(4):
                items.extend(v_group(sub))
            return items

        def norm_part1(ctx_a, ctx_b, j, pair, ctxn, fast=False):
            """Drain ctx psum (incl. the l rows at 64) and compute 1/l.
            Default path: repack both heads' l through DRAM onto all 128
            partitions so the DVE reciprocal costs ~200ns; the chain is all
            DVE+DMA and is consumed a full pair later (norm_part2), so the
            PE never waits on it. `fast=True` computes 1/l as exp(-ln(l)) on
            ScalarE instead — a short-latency chain for the final pair."""
            par = (j * NP + pair) % 2 * 2
            ra, rb = rec_t[par], rec_t[par + 1]
            with nc.allow_low_precision("bf16/fp32r attention pipeline"):
                cra = ctxraw_pool.tile([VW, 512], BF16, tag="cr",
                                       name=f"cra{j}{pair}")
                nc.vector.tensor_copy(cra[:], ctx_a[:])
                crb = ctxraw_pool.tile([VW, 512], BF16, tag="cr",
                                       name=f"crb{j}{pair}")
                nc.vector.tensor_copy(crb[:], ctx_b[:])
                if fast:
                    lrow = lrow_pool.tile([VW, 1024], F32, tag="lr",
                                          name=f"lr{j}{pair}")
                    nc.scalar.activation(
                        lrow[64:65, 0:512], ctx_a[64:65, :], AF.Ln)
                    nc.scalar.activation(
                        ra[64:65, :], lrow[64:65, 0:512],
                        AF.Exp, scale=-1.0)
                    nc.scalar.activation(
                        lrow[64:65, 512:1024], ctx_b[64:65, :], AF.Ln)
                    nc.scalar.activation(
                        rb[64:65, :], lrow[64:65, 512:1024],
                        AF.Exp, scale=-1.0)
                else:
                    lrow = lrow_pool.tile([VW, 1024], F32, tag="lr",
                                          name=f"lr{j}{pair}")
                    nc.vector.tensor_copy(
                        lrow[64:65, 0:512], ctx_a[64:65, :])
                    nc.vector.tensor_copy(
                        lrow[64:65, 512:1024], ctx_b[64:65, :])
                    ls = dram_pool.tile([1, 1024], F32, tag="ls",
                                        name=f"ls{j}{pair}")
                    nc.sync.dma_start(ls[:], lrow[64:65, :])
                    lpack = lpack_pool.tile([128, 8], F32, tag="lp",
                                            name=f"lp{j}{pair}")
                    nc.sync.dma_start(
                        lpack[:], ls[0].rearrange("(p f) -> p f", p=128))
                    rpk = lpack_pool.tile([128, 8], F32R, tag="rp",
                                          name=f"rp{j}{pair}")
                    nc.vector.reciprocal(rpk[:], lpack[:])
                    rs = dram_pool.tile([1, 1024], F32R, tag="rs",
                                        name=f"rs{j}{pair}")
                    nc.sync.dma_start(
                        rs[0].rearrange("(p f) -> p f", p=128), rpk[:])
                    nc.sync.dma_start(ra[64:65, :], rs[0:1, 0:512])
                    nc.sync.dma_start(rb[64:65, :], rs[0:1, 512:1024])
            return (ra, rb, cra, crb, j, pair, ctxn)

        def norm_part2(ra, rb, cra, crb, j, pair, ctxn):
            """Broadcast 1/l onto rows 0-63 with a tiny matmul (same PE
            tiling mode as the fills), multiply, and DMA head b's rows
            across to partitions 64-127."""
            with nc.allow_low_precision("bf16/fp32r attention pipeline"):
                cn = ctxn_pool.tile([128, 512], BF16, tag="cn",
                                    name=f"cn{j}{pair}")
                bps_a = mm_ps.tile([128, 512], F32, tag="mm",
                                   name=f"bpa{j}{pair}")
                nc.tensor.matmul(bps_a[:], bcA[:], ra[:],
                                 start=True, stop=True)
                nc.vector.tensor_mul(cn[0:64, :], cra[0:64, :],
                                     bps_a[0:64, :])
                bps_b = mm_ps.tile([128, 512], F32, tag="mm",
                                   name=f"bpb{j}{pair}")
                nc.tensor.matmul(bps_b[:], bcA[:], rb[:],
                                 start=True, stop=True)
                cnb = cnb_pool.tile([64, 512], BF16, tag="cnb",
                                    name=f"cnb{j}{pair}")
                nc.vector.tensor_mul(cnb[:], crb[0:64, :],
                                     bps_b[0:64, :])
                nc.gpsimd.dma_start(cn[64:128, :], cnb[:])
            ctxn.append(cn)

        def attention_block(j, fill, ctxn, carried):
            """Causal attention + softmax-denominator for q-block j.
            `fill` is a list of closures (projections / output projections)
            sprinkled into the PE stream to cover exp-wait stalls. `carried`
            is the previous pair's unfinished normalize (possibly from the
            previous block); the one left over here is returned."""
            fill = list(fill)
            n_triples = NP * 4 * (j + 1)
            per_triple = -(-len(fill) // n_triples) if fill else 0

            def emit_fill(n):
                for _ in range(n):
                    if not fill:
                        return
                    if fill[0]() is False:
                        return  # head item's inputs not produced yet
                    fill.pop(0)

            ni = 4 * (j + 1)

            def scores(pair, i):
                p = i - 4 * j
                lo = 128 * p if p > 0 else 0
                sc = sc_ps.tile([128, 1024], F32, tag="sc",
                                name=f"sc{j}{pair}{i}")
                qa = QT[pair][0:64, j * 512 + lo:(j + 1) * 512]
                qb = QT[pair][64:128, j * 512 + lo:(j + 1) * 512]
                ka = KTt[pair][0:64, i * 128:(i + 1) * 128]
                kb = KTt[pair][64:128, i * 128:(i + 1) * 128]
                nc.tensor.matmul(
                    sc[:, lo:512], ka, qa,
                    start=True, stop=True, tile_position=(0, 0),
                )
                nc.tensor.matmul(
                    sc[:, 512 + lo:1024], kb, qb,
                    start=True, stop=True, tile_position=(64, 0),
                )
                return sc

            def emit_exp(sc, i, pair):
                p = i - 4 * j
                et = exp_pool.tile([128, 1024], BF16, tag="exp",
                                   name=f"et{j}{pair}{i}")
                if p >= 2:
                    lo = 128 * p
                    nc.scalar.activation(
                        et[:, lo:512], sc[:, lo:512], AF.Exp)
                    nc.scalar.activation(
                        et[:, 512 + lo:1024], sc[:, 512 + lo:1024], AF.Exp)
                else:
                    nc.scalar.activation(et[:], sc[:], AF.Exp)
                if p >= 0:
                    # diagonal block: zero the future positions within the
                    # 128-wide triangle at [lo, lo+128): keep iff qq' >= kk
                    lo = 128 * p
                    ap = et[:].rearrange(
                        "p (h q) -> p h q", h=2)[:, :, lo:lo + 128]
                    nc.gpsimd.affine_select(
                        out=ap, in_=ap,
                        pattern=[[0, 2], [1, 128]],
                        compare_op=ALU.is_ge,
                        fill=0.0,
                        base=0,
                        channel_multiplier=-1,
                    )
                return et

            def emit_ctx(ctx_a, ctx_b, et, i):
                p = i - 4 * j
                lo = 128 * p if p > 0 else 0
                first, last = (i == 0), (i == ni - 1)
                va = V[i][:, pair * 2 * VW: pair * 2 * VW + VW]
                vb = V[i][:, pair * 2 * VW + VW: pair * 2 * VW + 2 * VW]
                nc.tensor.matmul(
                    ctx_a[:, lo:512], va, et[:, lo:512],
                    start=first, stop=last, skip_group_check=True,
                )
                nc.tensor.matmul(
                    ctx_b[:, lo:512], vb, et[:, 512 + lo:1024],
                    start=first, stop=last, skip_group_check=True,
                )

            for pair in range(NP):
                ctx_a = ctx_ps.tile([VW, 512], F32, tag="ctx",
                                    name=f"ctxa{j}{pair}")
                ctx_b = ctx_ps.tile([VW, 512], F32, tag="ctx",
                                    name=f"ctxb{j}{pair}")
                sc = scores(pair, 0)
                pending = None
                for i in range(ni):
                    et = emit_exp(sc, i, pair)
                    if i + 1 < ni:
                        sc = scores(pair, i + 1)
                    if pending is not None:
                        emit_ctx(ctx_a, ctx_b, *pending)
                        emit_fill(per_triple)
                    if carried is not None and i >= 1:
                        # a full pair has passed — the reciprocal repack
                        # chain is done, so the broadcast matmuls dispatch
                        # without stalling the PE
                        norm_part2(*carried)
                        carried = None
                    pending = (et, i)
                emit_ctx(ctx_a, ctx_b, *pending)
                emit_fill(per_triple)
                fast = (j == 3 and pair == NP - 1)
                carried = norm_part1(ctx_a, ctx_b, j, pair, ctxn, fast=fast)
            if j == 3:
                norm_part2(*carried)
                carried = None

            # drain any remaining fill (all inputs exist by block end)
            while fill:
                assert fill[0]() is not False
                fill.pop(0)
            return carried

        def outproj_items(j, ctxn):
            """Output projection for q-tile j as fine-grain fill items."""
            items = []

            def group(s4, oh, holder):
                def mk_mm(pair):
                    def go():
                        if len(ctxn) <= pair:
                            return False  # cn not normalized yet
                        if "ps" not in holder:
                            holder["ps"] = mm_ps.tile(
                                [128, 512], F32, tag="mm", name=f"yp{j}{s4}{oh}")
                        nc.tensor.matmul(
                            holder["ps"][:],
                            ctxn[pair][:, s4 * 128:(s4 + 1) * 128],
                            wo_t[:, pair * D + oh * 512:
                                 pair * D + (oh + 1) * 512],
                            start=(pair == 0),
                            stop=(pair == NP - 1),
                        )
                    return go

                def copy():
                    nc.vector.tensor_copy(
                        holder["yb"][:, oh * 512:(oh + 1) * 512], holder["ps"][:])
                    del holder["ps"]

                return [mk_mm(p) for p in range(NP)] + [copy]

            for s4 in range(4):
                srow = j * 4 + s4
                holder = {}

                def alloc_yb(holder=holder, s4=s4):
                    holder["yb"] = ybuf_pool.tile(
                        [128, D], BF16, tag="yb", name=f"yb{j}{s4}")

                items.append(alloc_yb)
                for oh in range(2):
                    items.extend(group(s4, oh, holder))

                def dma_out(holder=holder, srow=srow):
                    nc.sync.dma_start(
                        yout[srow * 128:(srow + 1) * 128, :], holder["yb"][:])

                items.append(dma_out)
            return items

        # chunk 0 projections run alone (Q first, matching weight-DMA
        # arrival order); attention block j then carries chunk j+1's
        # projections and block j-1's output projection as PE filler for its
        # exp-wait stalls; block 3 additionally self-fills with its own
        # output projection (enabled by the eager, PE-free normalize).
        items0 = proj_items(0, xts0)
        for item in items0[:4 * (KT8 + 1)]:   # Q groups
            item()
        # constants + V ones-columns: emitted here so the loads don't
        # delay the startup DMAs
        nc.sync.dma_start(bcA[:], bc65[:])
        for t in rec_t:
            nc.sync.dma_start(t[0:64, :], zer64[:])
        for s in range(NS):
            ones_ap = V[s][:].rearrange("p (h c) -> p h c", c=VW)[:, :, 64:65]
            nc.gpsimd.memset(ones_ap, 1.0)
        for item in items0[4 * (KT8 + 1):8 * (KT8 + 1)]:   # K groups
            item()
        for item in items0[8 * (KT8 + 1):]:   # V groups
            item()

        prev_out = []
        carried = None
        for j in range(4):
            fill = list(prev_out)
            if j + 1 < 4:
                xts = emit_xt_dmas(j + 1)
                fill = proj_items(j + 1, xts) + fill
            ctxn = []
            if j == 3:
                prev_out = outproj_items(j, ctxn)
                fill = fill + prev_out
                prev_out = []
            carried = attention_block(j, fill, ctxn, carried)
            if j < 3:
                prev_out = outproj_items(j, ctxn)
        for item in prev_out:
            item()

    _enforce_wait_limits(nc.m)
    return nc


_NC = None


def _get_nc():
    global _NC
    if _NC is None:
        _NC = build_nc()
    return _NC


def run(x, Wq, Wk, Wv, Wo, trace=False, trace_kwargs=None):
    """Returns (y, BassKernelResults)."""
    x = np.asarray(x, np.float32)
    scale = 1.0 / np.sqrt(DK)
    bc = np.zeros((65, 128), np.float32)
    bc[64, 0:64] = 1.0
    zer = np.zeros((64, 512), np.float32)
    in_maps = []
    for core in range(N_CORES):
        b, g = core // 2, core % 2
        cols = slice(g * HG, (g + 1) * HG)
        bf = ml_dtypes.bfloat16
        in_maps.append({
            "xT": np.ascontiguousarray(x[b].T).astype(bf),
            "wqT": np.ascontiguousarray(
                np.asarray(Wq, np.float32).T[:, cols] * scale).astype(bf),
            "wkT": np.ascontiguousarray(
                np.asarray(Wk, np.float32).T[:, cols]).astype(bf),
            "wvT": np.ascontiguousarray(
                np.asarray(Wv, np.float32).T[:, cols]).astype(bf),
            "woT": np.ascontiguousarray(
                np.asarray(Wo, np.float32).T[cols, :]).astype(bf),
            "bc65": bc,
            "zer64": zer,
        })
    kw = dict(trace_kwargs or {})
    res = run_bass_kernel_spmd(
        _get_nc(), in_maps, list(range(N_CORES)), trace=trace, **kw
    )
    y = np.empty((B, S, D), np.float32)
    for b in range(B):
        y[b] = (res.results[2 * b]["y"].astype(np.float32)
                + res.results[2 * b + 1]["y"].astype(np.float32))
    return y, res


def kernel(x, Wq, Wk, Wv, Wo):
    y, _ = run(x, Wq, Wk, Wv, Wo)
    return y


# revision 32
# speedup vs baseline: 1.2291x; 1.0303x over previous
"""Multi-head causal self-attention on 8 Trainium2 NeuronCores.

Problem: x [4, 2048, 1024], Wq/Wk/Wv/Wo [1024, 1024] (applied as x @ W.T),
16 heads, dk=64, causal softmax, output [4, 2048, 1024], all fp32.

Sharding: 8 cores = 4 batches x 2 head-groups (8 heads each).
Each core computes QKV projections for its 8 heads, streaming causal
attention, and a partial output projection (Wo row-split). The host adds
the two partial outputs per batch element.

Per-core layouts (chosen so NO on-device transposes are needed):
  xT  [1024, 2048]  = x[b].T          (host-transposed)
  wqT [1024, 512]   = (Wq/8).T cols for this head group (1/sqrt(dk) folded)
  wkT [1024, 512], wvT [1024, 512]
  woT [512, 1024]   = Wo[:, cols].T
  QT/KT on chip as [feat, seq] (head pairs stacked on partitions),
  V as [seq, 65*8] bf16 with a ones column appended per head, so the
  AV matmul (stationary [V | 1], 65 outputs) yields both the context
  rows AND the softmax denominator row in one pass, in the same PE
  tiling mode (128x128) as the projection fills. Scores tiles
  [k=128, q=512] per head pair are exp'ed on ScalarE into bf16; the
  causal mask is applied with affine_select on the idle GpSimd engine;
  1/l is broadcast across partitions with gpsimd partition_broadcast.

Causal-wedge (diagonal) tiles are narrowed to their valid q-range, and
projection s-chunks / output-projection blocks are interleaved in
program order so TensorE, ScalarE and GpSimd work concurrently.
"""

import ml_dtypes
import numpy as np

import concourse.bass as bass
import concourse.mybir as mybir
import concourse.tile as tile
from concourse.bass_utils import run_bass_kernel_spmd
from concourse.vector_clock import ScopedClock

F32 = mybir.dt.float32
F32R = mybir.dt.float32r
BF16 = mybir.dt.bfloat16
AF = mybir.ActivationFunctionType
ALU = mybir.AluOpType

B, S, D = 4, 2048, 1024
H = 16
DK = 64
N_CORES = 8
HG = 512          # head-group width (8 heads x 64)


# ---------------------------------------------------------------------------
# This walrus accepts at most 1 sem wait per instruction (2 for
# EventSemaphore). Tile emits more in two places; both are fixed up here by
# moving excess waits onto preceding instructions on the same engine.
# ---------------------------------------------------------------------------
def _split_drain_and_barrier(self, tick_clock, wait_clock):
    nc = self.nc
    probe = nc.sync.nop(nofuse=True, hint="tile_drain_waits")
    wait_clock.add_sem_waits(
        probe.ins, ScopedClock({None: tick_clock.global_clock})
    )
    si = probe.ins.sync_info
    waits = list(si.on_wait) if si is not None else []
    if len(waits) > 1:
        probe.ins.sync_info = mybir.SyncInfo(on_wait=[waits[0]], on_update=[])
        for w in waits[1:]:
            n = nc.sync.nop(nofuse=True, hint="tile_drain_waits")
            n.ins.sync_info = mybir.SyncInfo(on_wait=[w], on_update=[])
    nc.sync.drain()
    nc.all_engine_barrier()
    popped = nc._tile_sem_poison_stack.pop()
    assert popped is self._sem_poison
    nc.clear_and_free_semaphores(list(self.sems.allocated().values()))
    nc.all_engine_barrier()


tile.TileContext._drain_and_barrier = _split_drain_and_barrier

_wsplit_counter = [0]


def _enforce_wait_limits(m):
    for fn in m.functions:
        for bb in fn.blocks:
            out = []
            changed = False
            for inst in bb.instructions:
                si = inst.sync_info
                cap = 2 if isinstance(inst, mybir.InstEventSemaphore) else 1
                if si is not None and len(si.on_wait) > cap:
                    waits = list(si.on_wait)
                    keep, extra = waits[:cap], waits[cap:]
                    for i in range(0, len(extra), 2):
                        _wsplit_counter[0] += 1
                        out.append(mybir.InstEventSemaphore(
                            name=f"I-wsplit-{_wsplit_counter[0]}",
                            engine=inst.engine,
                            ins=[], outs=[],
                            sync_info=mybir.SyncInfo(
                                on_wait=extra[i:i + 2], on_update=[]),
                        ))
                    inst.sync_info = mybir.SyncInfo(
                        on_wait=keep, on_update=list(si.on_update))
                    changed = True
                out.append(inst)
            if changed:
                bb.instructions = out


def build_nc():
    nc = bass.Bass()

    xT = nc.declare_dram_parameter("xT", [D, S], BF16, isOutput=False)
    wqT = nc.declare_dram_parameter("wqT", [D, HG], BF16, isOutput=False)
    wkT = nc.declare_dram_parameter("wkT", [D, HG], BF16, isOutput=False)
    wvT = nc.declare_dram_parameter("wvT", [D, HG], BF16, isOutput=False)
    woT = nc.declare_dram_parameter("woT", [HG, D], BF16, isOutput=False)
    bc65 = nc.declare_dram_parameter("bc65", [65, 128], F32R, isOutput=False)
    zer64 = nc.declare_dram_parameter("zer64", [64, 512], F32R,
                                      isOutput=False)
    yout = nc.declare_dram_parameter("y", [S, D], BF16, isOutput=True)

    KT8 = D // 128   # contraction tiles for the projections
    NP = 4           # head pairs per core
    NS = S // 128    # seq tiles of 128
    VW = 65          # per-head V width in the augmented layout ([V | 1])

    from contextlib import ExitStack

    with tile.TileContext(nc) as tc, ExitStack() as ctx:
        ep = ctx.enter_context
        consts = ep(tc.tile_pool(name="consts", bufs=1))
        qt_pool = ep(tc.tile_pool(name="qt", bufs=1))
        kt_pool = ep(tc.tile_pool(name="kt", bufs=1))
        v_pool = ep(tc.tile_pool(name="v", bufs=1))
        wo_pool = ep(tc.tile_pool(name="wo", bufs=1))
        wq_pool = ep(tc.tile_pool(name="wq", bufs=1))
        wk_pool = ep(tc.tile_pool(name="wk", bufs=1))
        wv_pool = ep(tc.tile_pool(name="wv", bufs=1))
        xt_pool = ep(tc.tile_pool(name="xt", bufs=2))
        exp_pool = ep(tc.tile_pool(name="exp", bufs=6))
        ctxn_pool = ep(tc.tile_pool(name="ctxn", bufs=12))
        cnb_pool = ep(tc.tile_pool(name="cnb", bufs=2))
        ctxraw_pool = ep(tc.tile_pool(name="ctxraw", bufs=4))
        lrow_pool = ep(tc.tile_pool(name="lrow", bufs=2))
        lpack_pool = ep(tc.tile_pool(name="lpack", bufs=4))
        dram_pool = ep(tc.tile_pool(name="ldram", bufs=2, space="DRAM"))
        ybuf_pool = ep(tc.tile_pool(name="ybuf", bufs=2))
        mm_ps = ep(tc.tile_pool(name="mm_ps", bufs=2, space="PSUM"))
        sc_ps = ep(tc.tile_pool(name="sc_ps", bufs=2, space="PSUM"))
        ctx_ps = ep(tc.tile_pool(name="ctx_ps", bufs=2, space="PSUM"))

        QT = [qt_pool.tile([128, S], BF16, tag=f"qt{p}", name=f"QT{p}")
              for p in range(NP)]
        KTt = [kt_pool.tile([128, S], BF16, tag=f"kt{p}", name=f"KTt{p}")
               for p in range(NP)]
        # V_aug: per seq-tile, [128, 8*65]: per head 64 V columns + a ones
        # column, so the AV stationary [V | 1] produces ctx rows 0-63 and
        # the softmax denominator at row 64.
        V = [v_pool.tile([128, 8 * VW], BF16, tag=f"v{s}", name=f"V{s}")
             for s in range(NS)]

        # ---- weight/x DMAs: batched into a few large transfers so the
        # issuing queues don't serialize startup. The first Q matmuls only
        # wait for the first wq/x slices.
        # x chunk: one [128, 8*512] tile, kt-major (cols kt*512..kt*512+511).
        xT_v = xT.rearrange("(a p) s -> p a s", p=128)     # [128, 8, 2048]
        wq_v = wqT.rearrange("(a p) h -> p a h", p=128)    # [128, 8, 512]
        wk_v = wkT.rearrange("(a p) h -> p a h", p=128)
        wv_v = wvT.rearrange("(a p) h -> p a h", p=128)
        wo_v = woT.rearrange("(a p) d -> p a d", p=128)    # [128, 4, 1024]

        def emit_xt_dmas(st, split=1):
            t = xt_pool.tile([128, KT8 * 512], BF16, tag="xt", name=f"xt{st}")
            tv = t[:].rearrange("p (a s) -> p a s", s=512)
            step = KT8 // split
            for i in range(split):
                nc.gpsimd.dma_start(
                    tv[:, i * step:(i + 1) * step, :],
                    xT_v[:, i * step:(i + 1) * step,
                         st * 512:(st + 1) * 512])
            return t

        xts0 = emit_xt_dmas(0, split=4)
        wq_t = wq_pool.tile([128, KT8 * 512], BF16, tag="wq", name="wq")
        wqv = wq_t[:].rearrange("p (a h) -> p a h", h=512)
        nc.sync.dma_start(wqv[:, 0:4, :], wq_v[:, 0:4, :])
        nc.sync.dma_start(wqv[:, 4:8, :], wq_v[:, 4:8, :])
        wk_t = wk_pool.tile([128, KT8 * 512], BF16, tag="wk", name="wk")
        nc.sync.dma_start(
            wk_t[:].rearrange("p (a h) -> p a h", h=512), wk_v[:])
        wv_t = wv_pool.tile([128, KT8 * 512], BF16, tag="wv", name="wv")
        nc.sync.dma_start(
            wv_t[:].rearrange("p (a h) -> p a h", h=512), wv_v[:])
        wo_t = wo_pool.tile([128, NP * D], BF16, tag="wo", name="wo")
        nc.sync.dma_start(
            wo_t[:].rearrange("p (a d) -> p a d", d=D), wo_v[:])
        # broadcast constant: bcA.T @ m replicates m's row 64 onto rows 0-63
        bcA = consts.tile([VW, 128], F32R, tag="bcA", name="bcA")
        # static scratch rows for 1/l (row 64 live, rows 0-63 stay zero)
        rec_t = [consts.tile([VW, 512], F32R, tag=f"rec{i}", name=f"rec{i}")
                 for i in range(4)]

        def proj_items(st, xts):
            """QKV projection work for chunk st as a flat list of closures,
            one instruction each, so they can be sprinkled between attention
            stages at fine grain."""
            items = []

            def qk_group(ot, w_t, dst, name):
                holder = {}

                def mk_mm(kt):
                    def go():
                        if "ps" not in holder:
                            holder["ps"] = mm_ps.tile(
                                [128, 512], F32, tag="mm", name=name)
                        nc.tensor.matmul(
                            holder["ps"][:],
                            w_t[:, kt * 512 + ot * 128:
                                kt * 512 + (ot + 1) * 128],
                            xts[:, kt * 512:(kt + 1) * 512],
                            start=(kt == 0),
                            stop=(kt == KT8 - 1),
                        )
                    return go

                def copy():
                    nc.vector.tensor_copy(
                        dst[ot][:, st * 512:(st + 1) * 512], holder["ps"][:])

                return [mk_mm(kt) for kt in range(KT8)] + [copy]

            def v_group(sub):
                holder = {}

                def mk_mm(kt):
                    def go():
                        if "ps" not in holder:
                            holder["ps"] = mm_ps.tile(
                                [128, 512], F32, tag="mm", name=f"pv{st}{sub}")
                        nc.tensor.matmul(
                            holder["ps"][:],
                            xts[:, kt * 512 + sub * 128:
                                kt * 512 + (sub + 1) * 128],
                            wv_t[:, kt * 512:(kt + 1) * 512],
                            start=(kt == 0),
                            stop=(kt == KT8 - 1),
                        )
                    return go

                def copy():
                    dst = V[st * 4 + sub][:].rearrange(
                        "p (h c) -> p h c", c=VW)[:, :, 0:64]
                    src = holder["ps"][:].rearrange("p (h c) -> p h c", c=64)
                    nc.vector.tensor_copy(dst, src)

                return [mk_mm(kt) for kt in range(KT8)] + [copy]

            for ot in range(NP):
                items.extend(qk_group(ot, wq_t, QT, f"pq{st}{ot}"))
            for ot in range(NP):
                items.extend(qk_group(ot, wk_t, KTt, f"pk{st}{ot}"))
            for sub in range(4):
                items.extend(v_group(sub))
            return items

        def norm_part1(ctx_a, ctx_b, j, pair, ctxn, fast=False):
            """Drain ctx psum (incl. the l rows at 64) and compute 1/l.
            Default path: repack both heads' l through DRAM onto all 128
            partitions so the DVE reciprocal costs ~200ns; the chain is all
            DVE+DMA and is consumed a full pair later (norm_part2), so the
            PE never waits on it. `fast=True` computes 1/l as exp(-ln(l)) on
            ScalarE instead — a short-latency chain for the final pair."""
            par = (j * NP + pair) % 2 * 2
            ra, rb = rec_t[par], rec_t[par + 1]
            with nc.allow_low_precision("bf16/fp32r attention pipeline"):
                cra = ctxraw_pool.tile([VW, 512], BF16, tag="cr",
                                       name=f"cra{j}{pair}")
                nc.vector.tensor_copy(cra[:], ctx_a[:])
                crb = ctxraw_pool.tile([VW, 512], BF16, tag="cr",
                                       name=f"crb{j}{pair}")
                nc.vector.tensor_copy(crb[:], ctx_b[:])
                if fast:
                    lrow = lrow_pool.tile([VW, 1024], F32, tag="lr",
                                          name=f"lr{j}{pair}")
                    nc.scalar.activation(
                        lrow[64:65, 0:512], ctx_a[64:65, :], AF.Ln)
                    nc.scalar.activation(
                        ra[64:65, :], lrow[64:65, 0:512],
                        AF.Exp, scale=-1.0)
                    nc.scalar.activation(
                        lrow[64:65, 512:1024], ctx_b[64:65, :], AF.Ln)
                    nc.scalar.activation(
                        rb[64:65, :], lrow[64:65, 512:1024],
                        AF.Exp, scale=-1.0)
                else:
                    lrow = lrow_pool.tile([VW, 1024], F32, tag="lr",
                                          name=f"lr{j}{pair}")
                    nc.vector.tensor_copy(
                        lrow[64:65, 0:512], ctx_a[64:65, :])
                    nc.vector.tensor_copy(
                        lrow[64:65, 512:1024], ctx_b[64:65, :])
                    ls = dram_pool.tile([1, 1024], F32, tag="ls",
                                        name=f"ls{j}{pair}")
                    nc.sync.dma_start(ls[:], lrow[64:65, :])
                    lpack = lpack_pool.tile([128, 8], F32, tag="lp",
                                            name=f"lp{j}{pair}")
                    nc.sync.dma_start(
                        lpack[:], ls[0].rearrange("(p f) -> p f", p=128))
                    rpk = lpack_pool.tile([128, 8], F32R, tag="rp",
                                          name=f"rp{j}{pair}")
                    nc.vector.reciprocal(rpk[:], lpack[:])
                    rs = dram_pool.tile([1, 1024], F32R, tag="rs",
                                        name=f"rs{j}{pair}")
                    nc.sync.dma_start(
                        rs[0].rearrange("(p f) -> p f", p=128), rpk[:])
                    nc.sync.dma_start(ra[64:65, :], rs[0:1, 0:512])
                    nc.sync.dma_start(rb[64:65, :], rs[0:1, 512:1024])
            return (ra, rb, cra, crb, j, pair, ctxn)

        def norm_part2(ra, rb, cra, crb, j, pair, ctxn):
            """Broadcast 1/l onto rows 0-63 with a tiny matmul (same PE
            tiling mode as the fills), multiply, and DMA head b's rows
            across to partitions 64-127."""
            with nc.allow_low_precision("bf16/fp32r attention pipeline"):
                cn = ctxn_pool.tile([128, 512], BF16, tag="cn",
                                    name=f"cn{j}{pair}")
                bps_a = mm_ps.tile([128, 512], F32, tag="mm",
                                   name=f"bpa{j}{pair}")
                nc.tensor.matmul(bps_a[:], bcA[:], ra[:],
                                 start=True, stop=True)
                nc.vector.tensor_mul(cn[0:64, :], cra[0:64, :],
                                     bps_a[0:64, :])
                bps_b = mm_ps.tile([128, 512], F32, tag="mm",
                                   name=f"bpb{j}{pair}")
                nc.tensor.matmul(bps_b[:], bcA[:], rb[:],
                                 start=True, stop=True)
                cnb = cnb_pool.tile([64, 512], BF16, tag="cnb",
                                    name=f"cnb{j}{pair}")
                nc.vector.tensor_mul(cnb[:], crb[0:64, :],
                                     bps_b[0:64, :])
                nc.gpsimd.dma_start(cn[64:128, :], cnb[:])
            ctxn.append(cn)

        def attention_block(j, fill, ctxn, carried):
            """Causal attention + softmax-denominator for q-block j.
            `fill` is a list of closures (projections / output projections)
            sprinkled into the PE stream to cover exp-wait stalls. `carried`
            is the previous pair's unfinished normalize (possibly from the
            previous block); the one left over here is returned."""
            fill = list(fill)
            n_triples = NP * 4 * (j + 1)
            per_triple = -(-len(fill) // n_triples) if fill else 0

            def emit_fill(n):
                for _ in range(n):
                    if not fill:
                        return
                    if fill[0]() is False:
                        return  # head item's inputs not produced yet
                    fill.pop(0)

            ni = 4 * (j + 1)

            def scores(pair, i):
                p = i - 4 * j
                lo = 128 * p if p > 0 else 0
                sc = sc_ps.tile([128, 1024], F32, tag="sc",
                                name=f"sc{j}{pair}{i}")
                qa = QT[pair][0:64, j * 512 + lo:(j + 1) * 512]
                qb = QT[pair][64:128, j * 512 + lo:(j + 1) * 512]
                ka = KTt[pair][0:64, i * 128:(i + 1) * 128]
                kb = KTt[pair][64:128, i * 128:(i + 1) * 128]
                nc.tensor.matmul(
                    sc[:, lo:512], ka, qa,
                    start=True, stop=True, tile_position=(0, 0),
                )
                nc.tensor.matmul(
                    sc[:, 512 + lo:1024], kb, qb,
                    start=True, stop=True, tile_position=(64, 0),
                )
                return sc

            def emit_exp(sc, i, pair):
                p = i - 4 * j
                et = exp_pool.tile([128, 1024], BF16, tag="exp",
                                   name=f"et{j}{pair}{i}")
                if p >= 1:
                    # single contiguous ACT over the valid tail of both
                    # heads (the dead middle is cheaper than a 2nd ACT's
                    # fixed cost; garbage regions are never streamed)
                    lo = 128 * p
                    nc.scalar.activation(
                        et[:, lo:1024], sc[:, lo:1024], AF.Exp)
                else:
                    nc.scalar.activation(et[:], sc[:], AF.Exp)
                if p >= 0:
                    # diagonal block: zero the future positions within the
                    # 128-wide triangle at [lo, lo+128): keep iff qq' >= kk
                    lo = 128 * p
                    ap = et[:].rearrange(
                        "p (h q) -> p h q", h=2)[:, :, lo:lo + 128]
                    nc.gpsimd.affine_select(
                        out=ap, in_=ap,
                        pattern=[[0, 2], [1, 128]],
                        compare_op=ALU.is_ge,
                        fill=0.0,
                        base=0,
                        channel_multiplier=-1,
                    )
                return et

            def emit_ctx(ctx_a, ctx_b, et, i):
                p = i - 4 * j
                lo = 128 * p if p > 0 else 0
                first, last = (i == 0), (i == ni - 1)
                va = V[i][:, pair * 2 * VW: pair * 2 * VW + VW]
                vb = V[i][:, pair * 2 * VW + VW: pair * 2 * VW + 2 * VW]
                nc.tensor.matmul(
                    ctx_a[:, lo:512], va, et[:, lo:512],
                    start=first, stop=last, skip_group_check=True,
                )
                nc.tensor.matmul(
                    ctx_b[:, lo:512], vb, et[:, 512 + lo:1024],
                    start=first, stop=last, skip_group_check=True,
                )

            for pair in range(NP):
                ctx_a = ctx_ps.tile([VW, 512], F32, tag="ctx",
                                    name=f"ctxa{j}{pair}")
                ctx_b = ctx_ps.tile([VW, 512], F32, tag="ctx",
                                    name=f"ctxb{j}{pair}")
                # iterations run in PAIRS: one ctx batch + one scores batch
                # per super-iteration halves the PE tiling-mode switches
                sc0, sc1 = scores(pair, 0), scores(pair, 1)
                pend = []
                for i in range(0, ni, 2):
                    et0 = emit_exp(sc0, i, pair)
                    et1 = emit_exp(sc1, i + 1, pair)
                    if pend:
                        emit_ctx(ctx_a, ctx_b, *pend[0])
                        emit_ctx(ctx_a, ctx_b, *pend[1])
                        emit_fill(2 * per_triple)
                    if carried is not None and i >= 2:
                        # a full pair has passed — the reciprocal repack
                        # chain is done, so the broadcast matmuls dispatch
                        # without stalling the PE
                        norm_part2(*carried)
                        carried = None
                    if i + 2 < ni:
                        sc0 = scores(pair, i + 2)
                        sc1 = scores(pair, i + 3)
                    pend = [(et0, i), (et1, i + 1)]
                emit_ctx(ctx_a, ctx_b, *pend[0])
                emit_ctx(ctx_a, ctx_b, *pend[1])
                emit_fill(2 * per_triple)
                fast = (j == 3 and pair == NP - 1)
                carried = norm_part1(ctx_a, ctx_b, j, pair, ctxn, fast=fast)
            if j == 3:
                norm_part2(*carried)
                carried = None

            # drain any remaining fill (all inputs exist by block end)
            while fill:
                assert fill[0]() is not False
                fill.pop(0)
            return carried

        def outproj_items(j, ctxn):
            """Output projection for q-tile j as fine-grain fill items."""
            items = []

            def group(s4, oh, holder):
                def mk_mm(pair):
                    def go():
                        if len(ctxn) <= pair:
                            return False  # cn not normalized yet
                        if "ps" not in holder:
                            holder["ps"] = mm_ps.tile(
                                [128, 512], F32, tag="mm", name=f"yp{j}{s4}{oh}")
                        nc.tensor.matmul(
                            holder["ps"][:],
                            ctxn[pair][:, s4 * 128:(s4 + 1) * 128],
                            wo_t[:, pair * D + oh * 512:
                                 pair * D + (oh + 1) * 512],
                            start=(pair == 0),
                            stop=(pair == NP - 1),
                        )
                    return go

                def copy():
                    nc.vector.tensor_copy(
                        holder["yb"][:, oh * 512:(oh + 1) * 512], holder["ps"][:])
                    del holder["ps"]

                return [mk_mm(p) for p in range(NP)] + [copy]

            for s4 in range(4):
                srow = j * 4 + s4
                holder = {}

                def alloc_yb(holder=holder, s4=s4):
                    holder["yb"] = ybuf_pool.tile(
                        [128, D], BF16, tag="yb", name=f"yb{j}{s4}")

                items.append(alloc_yb)
                for oh in range(2):
                    items.extend(group(s4, oh, holder))

                def dma_out(holder=holder, srow=srow):
                    nc.sync.dma_start(
                        yout[srow * 128:(srow + 1) * 128, :], holder["yb"][:])

                items.append(dma_out)
            return items

        # chunk 0 projections run alone (Q first, matching weight-DMA
        # arrival order); attention block j then carries chunk j+1's
        # projections and block j-1's output projection as PE filler for its
        # exp-wait stalls; block 3 additionally self-fills with its own
        # output projection (enabled by the eager, PE-free normalize).
        items0 = proj_items(0, xts0)
        for item in items0[:4 * (KT8 + 1)]:   # Q groups
            item()
        # constants + V ones-columns: emitted here so the loads don't
        # delay the startup DMAs
        nc.sync.dma_start(bcA[:], bc65[:])
        for t in rec_t:
            nc.sync.dma_start(t[0:64, :], zer64[:])
        for s in range(NS):
            ones_ap = V[s][:].rearrange("p (h c) -> p h c", c=VW)[:, :, 64:65]
            nc.gpsimd.memset(ones_ap, 1.0)
        for item in items0[4 * (KT8 + 1):8 * (KT8 + 1)]:   # K groups
            item()
        for item in items0[8 * (KT8 + 1):]:   # V groups
            item()

        prev_out = []
        carried = None
        for j in range(4):
            fill = list(prev_out)
            if j + 1 < 4:
                xts = emit_xt_dmas(j + 1)
                fill = proj_items(j + 1, xts) + fill
            ctxn = []
            if j == 3:
                prev_out = outproj_items(j, ctxn)
                fill = fill + prev_out
                prev_out = []
            carried = attention_block(j, fill, ctxn, carried)
            if j < 3:
                prev_out = outproj_items(j, ctxn)
        for item in prev_out:
            item()

    _enforce_wait_limits(nc.m)
    return nc


_NC = None


def _get_nc():
    global _NC
    if _NC is None:
        _NC = build_nc()
    return _NC


def run(x, Wq, Wk, Wv, Wo, trace=False, trace_kwargs=None):
    """Returns (y, BassKernelResults)."""
    x = np.asarray(x, np.float32)
    scale = 1.0 / np.sqrt(DK)
    bc = np.zeros((65, 128), np.float32)
    bc[64, 0:64] = 1.0
    zer = np.zeros((64, 512), np.float32)
    in_maps = []
    for core in range(N_CORES):
        b, g = core // 2, core % 2
        cols = slice(g * HG, (g + 1) * HG)
        bf = ml_dtypes.bfloat16
        in_maps.append({
            "xT": np.ascontiguousarray(x[b].T).astype(bf),
            "wqT": np.ascontiguousarray(
                np.asarray(Wq, np.float32).T[:, cols] * scale).astype(bf),
            "wkT": np.ascontiguousarray(
                np.asarray(Wk, np.float32).T[:, cols]).astype(bf),
            "wvT": np.ascontiguousarray(
                np.asarray(Wv, np.float32).T[:, cols]).astype(bf),
            "woT": np.ascontiguousarray(
                np.asarray(Wo, np.float32).T[cols, :]).astype(bf),
            "bc65": bc,
            "zer64": zer,
        })
    kw = dict(trace_kwargs or {})
    res = run_bass_kernel_spmd(
        _get_nc(), in_maps, list(range(N_CORES)), trace=trace, **kw
    )
    y = np.empty((B, S, D), np.float32)
    for b in range(B):
        y[b] = (res.results[2 * b]["y"].astype(np.float32)
                + res.results[2 * b + 1]["y"].astype(np.float32))
    return y, res


def kernel(x, Wq, Wk, Wv, Wo):
    y, _ = run(x, Wq, Wk, Wv, Wo)
    return y


# revision 39
# speedup vs baseline: 1.2639x; 1.0283x over previous
"""Multi-head causal self-attention on 8 Trainium2 NeuronCores.

Problem: x [4, 2048, 1024], Wq/Wk/Wv/Wo [1024, 1024] (applied as x @ W.T),
16 heads, dk=64, causal softmax, output [4, 2048, 1024], all fp32.

Sharding: 8 cores = 4 batches x 2 head-groups (8 heads each).
Each core computes QKV projections for its 8 heads, streaming causal
attention, and a partial output projection (Wo row-split). The host adds
the two partial outputs per batch element.

Per-core layouts (chosen so NO on-device transposes are needed):
  xT  [1024, 2048]  = x[b].T          (host-transposed)
  wqT [1024, 512]   = (Wq/8).T cols for this head group (1/sqrt(dk) folded)
  wkT [1024, 512], wvT [1024, 512]
  woT [512, 1024]   = Wo[:, cols].T
  QT/KT on chip as [feat, seq] (head pairs stacked on partitions),
  V as [seq, 65*8] bf16 with a ones column appended per head, so the
  AV matmul (stationary [V | 1], 65 outputs) yields both the context
  rows AND the softmax denominator row in one pass, in the same PE
  tiling mode (128x128) as the projection fills. Scores tiles
  [k=128, q=512] per head pair are exp'ed on ScalarE into bf16; the
  causal mask is applied with affine_select on the idle GpSimd engine;
  1/l is broadcast across partitions with gpsimd partition_broadcast.

Causal-wedge (diagonal) tiles are narrowed to their valid q-range, and
projection s-chunks / output-projection blocks are interleaved in
program order so TensorE, ScalarE and GpSimd work concurrently.
"""

import ml_dtypes
import numpy as np

import concourse.bass as bass
import concourse.mybir as mybir
import concourse.tile as tile
from concourse.bass_utils import run_bass_kernel_spmd
from concourse.vector_clock import ScopedClock

F32 = mybir.dt.float32
F32R = mybir.dt.float32r
BF16 = mybir.dt.bfloat16
AF = mybir.ActivationFunctionType
ALU = mybir.AluOpType

B, S, D = 4, 2048, 1024
H = 16
DK = 64
N_CORES = 8
HG = 512          # head-group width (8 heads x 64)


# ---------------------------------------------------------------------------
# This walrus accepts at most 1 sem wait per instruction (2 for
# EventSemaphore). Tile emits more in two places; both are fixed up here by
# moving excess waits onto preceding instructions on the same engine.
# ---------------------------------------------------------------------------
def _split_drain_and_barrier(self, tick_clock, wait_clock):
    nc = self.nc
    probe = nc.sync.nop(nofuse=True, hint="tile_drain_waits")
    wait_clock.add_sem_waits(
        probe.ins, ScopedClock({None: tick_clock.global_clock})
    )
    si = probe.ins.sync_info
    waits = list(si.on_wait) if si is not None else []
    if len(waits) > 1:
        probe.ins.sync_info = mybir.SyncInfo(on_wait=[waits[0]], on_update=[])
        for w in waits[1:]:
            n = nc.sync.nop(nofuse=True, hint="tile_drain_waits")
            n.ins.sync_info = mybir.SyncInfo(on_wait=[w], on_update=[])
    nc.sync.drain()
    nc.all_engine_barrier()
    popped = nc._tile_sem_poison_stack.pop()
    assert popped is self._sem_poison
    nc.clear_and_free_semaphores(list(self.sems.allocated().values()))
    nc.all_engine_barrier()


tile.TileContext._drain_and_barrier = _split_drain_and_barrier

_wsplit_counter = [0]


def _enforce_wait_limits(m):
    for fn in m.functions:
        for bb in fn.blocks:
            out = []
            changed = False
            for inst in bb.instructions:
                si = inst.sync_info
                cap = 2 if isinstance(inst, mybir.InstEventSemaphore) else 1
                if si is not None and len(si.on_wait) > cap:
                    waits = list(si.on_wait)
                    keep, extra = waits[:cap], waits[cap:]
                    for i in range(0, len(extra), 2):
                        _wsplit_counter[0] += 1
                        out.append(mybir.InstEventSemaphore(
                            name=f"I-wsplit-{_wsplit_counter[0]}",
                            engine=inst.engine,
                            ins=[], outs=[],
                            sync_info=mybir.SyncInfo(
                                on_wait=extra[i:i + 2], on_update=[]),
                        ))
                    inst.sync_info = mybir.SyncInfo(
                        on_wait=keep, on_update=list(si.on_update))
                    changed = True
                out.append(inst)
            if changed:
                bb.instructions = out


def build_nc():
    nc = bass.Bass()

    xT = nc.declare_dram_parameter("xT", [D, S], BF16, isOutput=False)
    wqT = nc.declare_dram_parameter("wqT", [D, HG], BF16, isOutput=False)
    wkT = nc.declare_dram_parameter("wkT", [D, HG], BF16, isOutput=False)
    wvT = nc.declare_dram_parameter("wvT", [D, HG], BF16, isOutput=False)
    woT = nc.declare_dram_parameter("woT", [HG, D], BF16, isOutput=False)
    bc65 = nc.declare_dram_parameter("bc65", [65, 128], F32R, isOutput=False)
    zer64 = nc.declare_dram_parameter("zer64", [64, 512], F32R,
                                      isOutput=False)
    yout = nc.declare_dram_parameter("y", [S, D], BF16, isOutput=True)

    KT8 = D // 128   # contraction tiles for the projections
    NP = 4           # head pairs per core
    NS = S // 128    # seq tiles of 128
    VW = 65          # per-head V width in the augmented layout ([V | 1])

    from contextlib import ExitStack

    with tile.TileContext(nc) as tc, ExitStack() as ctx:
        ep = ctx.enter_context
        consts = ep(tc.tile_pool(name="consts", bufs=1))
        qt_pool = ep(tc.tile_pool(name="qt", bufs=1))
        kt_pool = ep(tc.tile_pool(name="kt", bufs=1))
        v_pool = ep(tc.tile_pool(name="v", bufs=1))
        wo_pool = ep(tc.tile_pool(name="wo", bufs=1))
        wq_pool = ep(tc.tile_pool(name="wq", bufs=1))
        wk_pool = ep(tc.tile_pool(name="wk", bufs=1))
        wv_pool = ep(tc.tile_pool(name="wv", bufs=1))
        xt_pool = ep(tc.tile_pool(name="xt", bufs=2))
        exp_pool = ep(tc.tile_pool(name="exp", bufs=6))
        ctxn_pool = ep(tc.tile_pool(name="ctxn", bufs=12))
        cnb_pool = ep(tc.tile_pool(name="cnb", bufs=2))
        ctxraw_pool = ep(tc.tile_pool(name="ctxraw", bufs=4))
        lrow_pool = ep(tc.tile_pool(name="lrow", bufs=2))
        lpack_pool = ep(tc.tile_pool(name="lpack", bufs=4))
        dram_pool = ep(tc.tile_pool(name="ldram", bufs=2, space="DRAM"))
        ybuf_pool = ep(tc.tile_pool(name="ybuf", bufs=2))
        mm_ps = ep(tc.tile_pool(name="mm_ps", bufs=2, space="PSUM"))
        sc_ps = ep(tc.tile_pool(name="sc_ps", bufs=2, space="PSUM"))
        ctx_ps = ep(tc.tile_pool(name="ctx_ps", bufs=2, space="PSUM"))

        QT = [qt_pool.tile([128, S], BF16, tag=f"qt{p}", name=f"QT{p}")
              for p in range(NP)]
        KTt = [kt_pool.tile([128, S], BF16, tag=f"kt{p}", name=f"KTt{p}")
               for p in range(NP)]
        # V_aug: per seq-tile, [128, 8*65]: per head 64 V columns + a ones
        # column, so the AV stationary [V | 1] produces ctx rows 0-63 and
        # the softmax denominator at row 64.
        V = [v_pool.tile([128, 8 * VW], BF16, tag=f"v{s}", name=f"V{s}")
             for s in range(NS)]

        # ---- weight/x DMAs: batched into a few large transfers so the
        # issuing queues don't serialize startup. The first Q matmuls only
        # wait for the first wq/x slices.
        # x chunk: one [128, 8*512] tile, kt-major (cols kt*512..kt*512+511).
        xT_v = xT.rearrange("(a p) s -> p a s", p=128)     # [128, 8, 2048]
        wq_v = wqT.rearrange("(a p) h -> p a h", p=128)    # [128, 8, 512]
        wk_v = wkT.rearrange("(a p) h -> p a h", p=128)
        wv_v = wvT.rearrange("(a p) h -> p a h", p=128)
        wo_v = woT.rearrange("(a p) d -> p a d", p=128)    # [128, 4, 1024]

        def emit_xt_dmas(st, split=1):
            t = xt_pool.tile([128, KT8 * 512], BF16, tag="xt", name=f"xt{st}")
            tv = t[:].rearrange("p (a s) -> p a s", s=512)
            step = KT8 // split
            for i in range(split):
                nc.gpsimd.dma_start(
                    tv[:, i * step:(i + 1) * step, :],
                    xT_v[:, i * step:(i + 1) * step,
                         st * 512:(st + 1) * 512])
            return t

        xts0 = emit_xt_dmas(0, split=4)
        wq_t = wq_pool.tile([128, KT8 * 512], BF16, tag="wq", name="wq")
        wqv = wq_t[:].rearrange("p (a h) -> p a h", h=512)
        nc.sync.dma_start(wqv[:, 0:4, :], wq_v[:, 0:4, :])
        nc.sync.dma_start(wqv[:, 4:8, :], wq_v[:, 4:8, :])
        wk_t = wk_pool.tile([128, KT8 * 512], BF16, tag="wk", name="wk")
        nc.sync.dma_start(
            wk_t[:].rearrange("p (a h) -> p a h", h=512), wk_v[:])
        wv_t = wv_pool.tile([128, KT8 * 512], BF16, tag="wv", name="wv")
        nc.sync.dma_start(
            wv_t[:].rearrange("p (a h) -> p a h", h=512), wv_v[:])
        wo_t = wo_pool.tile([128, NP * D], BF16, tag="wo", name="wo")
        nc.sync.dma_start(
            wo_t[:].rearrange("p (a d) -> p a d", d=D), wo_v[:])
        # broadcast constant: bcA.T @ m replicates m's row 64 onto rows 0-63
        bcA = consts.tile([VW, 128], F32R, tag="bcA", name="bcA")
        # static scratch rows for 1/l (row 64 live, rows 0-63 stay zero)
        rec_t = [consts.tile([VW, 512], F32R, tag=f"rec{i}", name=f"rec{i}")
                 for i in range(4)]

        def proj_items(st, xts):
            """QKV projection work for chunk st as a flat list of closures,
            one instruction each, so they can be sprinkled between attention
            stages at fine grain."""
            items = []

            def qk_group(ot, w_t, dst, name):
                holder = {}

                def mk_mm(kt):
                    def go():
                        if "ps" not in holder:
                            holder["ps"] = mm_ps.tile(
                                [128, 512], F32, tag="mm", name=name)
                        nc.tensor.matmul(
                            holder["ps"][:],
                            w_t[:, kt * 512 + ot * 128:
                                kt * 512 + (ot + 1) * 128],
                            xts[:, kt * 512:(kt + 1) * 512],
                            start=(kt == 0),
                            stop=(kt == KT8 - 1),
                        )
                    return go

                def copy():
                    nc.vector.tensor_copy(
                        dst[ot][:, st * 512:(st + 1) * 512], holder["ps"][:])

                return [mk_mm(kt) for kt in range(KT8)] + [copy]

            def v_group(sub):
                holder = {}

                def mk_mm(kt):
                    def go():
                        if "ps" not in holder:
                            holder["ps"] = mm_ps.tile(
                                [128, 512], F32, tag="mm", name=f"pv{st}{sub}")
                        nc.tensor.matmul(
                            holder["ps"][:],
                            xts[:, kt * 512 + sub * 128:
                                kt * 512 + (sub + 1) * 128],
                            wv_t[:, kt * 512:(kt + 1) * 512],
                            start=(kt == 0),
                            stop=(kt == KT8 - 1),
                        )
                    return go

                def copy():
                    dst = V[st * 4 + sub][:].rearrange(
                        "p (h c) -> p h c", c=VW)[:, :, 0:64]
                    src = holder["ps"][:].rearrange("p (h c) -> p h c", c=64)
                    nc.vector.tensor_copy(dst, src)

                return [mk_mm(kt) for kt in range(KT8)] + [copy]

            for ot in range(NP):
                items.extend(qk_group(ot, wq_t, QT, f"pq{st}{ot}"))
            for ot in range(NP):
                items.extend(qk_group(ot, wk_t, KTt, f"pk{st}{ot}"))
            for sub in range(4):
                items.extend(v_group(sub))
            return items

        def norm_part1(ctx_a, ctx_b, j, pair, ctxn, fast=False):
            """Drain ctx psum (incl. the l rows at 64) and compute 1/l.
            Default path: repack both heads' l through DRAM onto all 128
            partitions so the DVE reciprocal costs ~200ns; the chain is all
            DVE+DMA and is consumed a full pair later (norm_part2), so the
            PE never waits on it. `fast=True` computes 1/l as exp(-ln(l)) on
            ScalarE instead — a short-latency chain for the final pair."""
            par = (j * NP + pair) % 2 * 2
            ra, rb = rec_t[par], rec_t[par + 1]
            with nc.allow_low_precision("bf16/fp32r attention pipeline"):
                cra = ctxraw_pool.tile([VW, 512], BF16, tag="cr",
                                       name=f"cra{j}{pair}")
                nc.vector.tensor_copy(cra[:], ctx_a[:])
                crb = ctxraw_pool.tile([VW, 512], BF16, tag="cr",
                                       name=f"crb{j}{pair}")
                nc.vector.tensor_copy(crb[:], ctx_b[:])
                if fast:
                    lrow = lrow_pool.tile([VW, 1024], F32, tag="lr",
                                          name=f"lr{j}{pair}")
                    nc.scalar.activation(
                        lrow[64:65, 0:512], ctx_a[64:65, :], AF.Ln)
                    nc.scalar.activation(
                        ra[64:65, :], lrow[64:65, 0:512],
                        AF.Exp, scale=-1.0)
                    nc.scalar.activation(
                        lrow[64:65, 512:1024], ctx_b[64:65, :], AF.Ln)
                    nc.scalar.activation(
                        rb[64:65, :], lrow[64:65, 512:1024],
                        AF.Exp, scale=-1.0)
                else:
                    lrow = lrow_pool.tile([VW, 1024], F32, tag="lr",
                                          name=f"lr{j}{pair}")
                    nc.vector.tensor_copy(
                        lrow[64:65, 0:512], ctx_a[64:65, :])
                    nc.vector.tensor_copy(
                        lrow[64:65, 512:1024], ctx_b[64:65, :])
                    ls = dram_pool.tile([1, 1024], F32, tag="ls",
                                        name=f"ls{j}{pair}")
                    nc.sync.dma_start(ls[:], lrow[64:65, :])
                    lpack = lpack_pool.tile([128, 8], F32, tag="lp",
                                            name=f"lp{j}{pair}")
                    nc.sync.dma_start(
                        lpack[:], ls[0].rearrange("(p f) -> p f", p=128))
                    rpk = lpack_pool.tile([128, 8], F32R, tag="rp",
                                          name=f"rp{j}{pair}")
                    nc.vector.reciprocal(rpk[:], lpack[:])
                    rs = dram_pool.tile([1, 1024], F32R, tag="rs",
                                        name=f"rs{j}{pair}")
                    nc.sync.dma_start(
                        rs[0].rearrange("(p f) -> p f", p=128), rpk[:])
                    nc.sync.dma_start(ra[64:65, :], rs[0:1, 0:512])
                    nc.sync.dma_start(rb[64:65, :], rs[0:1, 512:1024])
            return (ra, rb, cra, crb, j, pair, ctxn)

        def norm_part2(ra, rb, cra, crb, j, pair, ctxn):
            """Broadcast 1/l onto rows 0-63 with a tiny matmul (same PE
            tiling mode as the fills), multiply, and DMA head b's rows
            across to partitions 64-127."""
            with nc.allow_low_precision("bf16/fp32r attention pipeline"):
                cn = ctxn_pool.tile([128, 512], BF16, tag="cn",
                                    name=f"cn{j}{pair}")
                bps_a = mm_ps.tile([128, 512], F32, tag="mm",
                                   name=f"bpa{j}{pair}")
                nc.tensor.matmul(bps_a[:], bcA[:], ra[:],
                                 start=True, stop=True)
                nc.vector.tensor_mul(cn[0:64, :], cra[0:64, :],
                                     bps_a[0:64, :])
                bps_b = mm_ps.tile([128, 512], F32, tag="mm",
                                   name=f"bpb{j}{pair}")
                nc.tensor.matmul(bps_b[:], bcA[:], rb[:],
                                 start=True, stop=True)
                cnb = cnb_pool.tile([64, 512], BF16, tag="cnb",
                                    name=f"cnb{j}{pair}")
                nc.vector.tensor_mul(cnb[:], crb[0:64, :],
                                     bps_b[0:64, :])
                nc.gpsimd.dma_start(cn[64:128, :], cnb[:])
            ctxn.append(cn)

        def attention_block(j, fill, ctxn, carried):
            """Causal attention + softmax-denominator for q-block j.
            `fill` is a list of closures (projections / output projections)
            sprinkled into the PE stream to cover exp-wait stalls. `carried`
            is the previous pair's unfinished normalize (possibly from the
            previous block); the one left over here is returned."""
            fill = list(fill)
            n_super = NP * 4 * (j + 1) // 2
            n_fill0 = len(fill)
            state = {"k": 0, "done": 0}

            def emit_fill():
                # evenly pace the fills across the block's super-iterations
                state["k"] += 1
                target = n_fill0 * state["k"] // n_super
                while state["done"] < target and fill:
                    if fill[0]() is False:
                        return  # head item's inputs not produced yet
                    fill.pop(0)
                    state["done"] += 1

            ni = 4 * (j + 1)

            def scores(pair, i):
                p = i - 4 * j
                lo = 128 * p if p > 0 else 0
                sc = sc_ps.tile([128, 1024], F32, tag="sc",
                                name=f"sc{j}{pair}{i}")
                qa = QT[pair][0:64, j * 512 + lo:(j + 1) * 512]
                qb = QT[pair][64:128, j * 512 + lo:(j + 1) * 512]
                ka = KTt[pair][0:64, i * 128:(i + 1) * 128]
                kb = KTt[pair][64:128, i * 128:(i + 1) * 128]
                nc.tensor.matmul(
                    sc[:, lo:512], ka, qa,
                    start=True, stop=True, tile_position=(0, 0),
                )
                nc.tensor.matmul(
                    sc[:, 512 + lo:1024], kb, qb,
                    start=True, stop=True, tile_position=(64, 0),
                )
                return sc

            def emit_exp(sc, i, pair):
                p = i - 4 * j
                et = exp_pool.tile([128, 1024], BF16, tag="exp",
                                   name=f"et{j}{pair}{i}")
                if p >= 1:
                    # single contiguous ACT over the valid tail of both
                    # heads (the dead middle is cheaper than a 2nd ACT's
                    # fixed cost; garbage regions are never streamed)
                    lo = 128 * p
                    nc.scalar.activation(
                        et[:, lo:1024], sc[:, lo:1024], AF.Exp)
                else:
                    nc.scalar.activation(et[:], sc[:], AF.Exp)
                if p >= 0:
                    # diagonal block: zero the future positions within the
                    # 128-wide triangle at [lo, lo+128): keep iff qq' >= kk
                    lo = 128 * p
                    ap = et[:].rearrange(
                        "p (h q) -> p h q", h=2)[:, :, lo:lo + 128]
                    nc.gpsimd.affine_select(
                        out=ap, in_=ap,
                        pattern=[[0, 2], [1, 128]],
                        compare_op=ALU.is_ge,
                        fill=0.0,
                        base=0,
                        channel_multiplier=-1,
                    )
                return et

            def emit_ctx(ctx_a, ctx_b, et, i):
                p = i - 4 * j
                lo = 128 * p if p > 0 else 0
                first, last = (i == 0), (i == ni - 1)
                va = V[i][:, pair * 2 * VW: pair * 2 * VW + VW]
                vb = V[i][:, pair * 2 * VW + VW: pair * 2 * VW + 2 * VW]
                nc.tensor.matmul(
                    ctx_a[:, lo:512], va, et[:, lo:512],
                    start=first, stop=last, skip_group_check=True,
                )
                nc.tensor.matmul(
                    ctx_b[:, lo:512], vb, et[:, 512 + lo:1024],
                    start=first, stop=last, skip_group_check=True,
                )

            for pair in range(NP):
                ctx_a = ctx_ps.tile([VW, 512], F32, tag="ctx",
                                    name=f"ctxa{j}{pair}")
                ctx_b = ctx_ps.tile([VW, 512], F32, tag="ctx",
                                    name=f"ctxb{j}{pair}")
                # iterations run in PAIRS: one ctx batch + one scores batch
                # per super-iteration halves the PE tiling-mode switches
                sc0, sc1 = scores(pair, 0), scores(pair, 1)
                pend = []
                for i in range(0, ni, 2):
                    et0 = emit_exp(sc0, i, pair)
                    et1 = emit_exp(sc1, i + 1, pair)
                    if pend:
                        emit_ctx(ctx_a, ctx_b, *pend[0])
                        emit_ctx(ctx_a, ctx_b, *pend[1])
                        emit_fill()
                    if carried is not None and i >= 2:
                        # a full pair has passed — the reciprocal repack
                        # chain is done, so the broadcast matmuls dispatch
                        # without stalling the PE
                        norm_part2(*carried)
                        carried = None
                    if i + 2 < ni:
                        sc0 = scores(pair, i + 2)
                        sc1 = scores(pair, i + 3)
                    pend = [(et0, i), (et1, i + 1)]
                emit_ctx(ctx_a, ctx_b, *pend[0])
                emit_ctx(ctx_a, ctx_b, *pend[1])
                emit_fill()
                fast = (j == 3 and pair == NP - 1)
                carried = norm_part1(ctx_a, ctx_b, j, pair, ctxn, fast=fast)
            if j == 3:
                norm_part2(*carried)
                carried = None

            # drain any remaining fill (all inputs exist by block end)
            while fill:
                assert fill[0]() is not False
                fill.pop(0)
            return carried

        def outproj_items(j, ctxn, s4s=(0, 1, 2, 3)):
            """Output projection for q-tile j as fine-grain fill items."""
            items = []

            def group(s4, oh, holder):
                def mk_mm(pair):
                    def go():
                        if len(ctxn) <= pair:
                            return False  # cn not normalized yet
                        if "ps" not in holder:
                            holder["ps"] = mm_ps.tile(
                                [128, 512], F32, tag="mm", name=f"yp{j}{s4}{oh}")
                        nc.tensor.matmul(
                            holder["ps"][:],
                            ctxn[pair][:, s4 * 128:(s4 + 1) * 128],
                            wo_t[:, pair * D + oh * 512:
                                 pair * D + (oh + 1) * 512],
                            start=(pair == 0),
                            stop=(pair == NP - 1),
                        )
                    return go

                def copy():
                    nc.vector.tensor_copy(
                        holder["yb"][:, oh * 512:(oh + 1) * 512], holder["ps"][:])
                    del holder["ps"]

                return [mk_mm(p) for p in range(NP)] + [copy]

            for s4 in s4s:
                srow = j * 4 + s4
                holder = {}

                def alloc_yb(holder=holder, s4=s4):
                    holder["yb"] = ybuf_pool.tile(
                        [128, D], BF16, tag="yb", name=f"yb{j}{s4}")

                items.append(alloc_yb)
                for oh in range(2):
                    items.extend(group(s4, oh, holder))

                def dma_out(holder=holder, srow=srow, j=j):
                    eng = nc.gpsimd if j == 3 else nc.sync
                    eng.dma_start(
                        yout[srow * 128:(srow + 1) * 128, :], holder["yb"][:])

                items.append(dma_out)
            return items

        # chunk 0 projections run alone (Q first, matching weight-DMA
        # arrival order); attention block j then carries chunk j+1's
        # projections and block j-1's output projection as PE filler for its
        # exp-wait stalls; block 3 additionally self-fills with its own
        # output projection (enabled by the eager, PE-free normalize).
        items0 = proj_items(0, xts0)
        for item in items0[:4 * (KT8 + 1)]:   # Q groups
            item()
        # constants + V ones-columns: emitted here so the loads don't
        # delay the startup DMAs
        nc.sync.dma_start(bcA[:], bc65[:])
        for t in rec_t:
            nc.sync.dma_start(t[0:64, :], zer64[:])
        for s in range(NS):
            ones_ap = V[s][:].rearrange("p (h c) -> p h c", c=VW)[:, :, 64:65]
            nc.gpsimd.memset(ones_ap, 1.0)
        for item in items0[4 * (KT8 + 1):8 * (KT8 + 1)]:   # K groups
            item()
        for item in items0[8 * (KT8 + 1):]:   # V groups
            item()

        # fills per block, balanced against each block's exp-paced slack:
        # block j gets chunk j+1's projections, the previous block's first
        # outproj half, the block-before's second half, and block 3
        # self-fills with its own outproj (gated on cn availability).
        ctxns = {}
        halfA, halfB = [], []
        carried = None
        for j in range(4):
            fill = halfB + halfA
            halfA, halfB = [], []
            if j + 1 < 4:
                xts = emit_xt_dmas(j + 1)
                fill = proj_items(j + 1, xts) + fill
            ctxn = []
            ctxns[j] = ctxn
            if j > 0:
                halfB = outproj_items(j - 1, ctxns[j - 1], s4s=(2, 3))
            if j == 3:
                fill = fill + halfB + outproj_items(j, ctxn)
                halfB = []
            carried = attention_block(j, fill, ctxn, carried)
            if j < 3:
                halfA = outproj_items(j, ctxn, s4s=(0, 1))
        assert not halfA and not halfB

    _enforce_wait_limits(nc.m)
    return nc


_NC = None


def _get_nc():
    global _NC
    if _NC is None:
        _NC = build_nc()
    return _NC


def run(x, Wq, Wk, Wv, Wo, trace=False, trace_kwargs=None):
    """Returns (y, BassKernelResults)."""
    x = np.asarray(x, np.float32)
    scale = 1.0 / np.sqrt(DK)
    bc = np.zeros((65, 128), np.float32)
    bc[64, 0:64] = 1.0
    zer = np.zeros((64, 512), np.float32)
    in_maps = []
    for core in range(N_CORES):
        b, g = core // 2, core % 2
        cols = slice(g * HG, (g + 1) * HG)
        bf = ml_dtypes.bfloat16
        in_maps.append({
            "xT": np.ascontiguousarray(x[b].T).astype(bf),
            "wqT": np.ascontiguousarray(
                np.asarray(Wq, np.float32).T[:, cols] * scale).astype(bf),
            "wkT": np.ascontiguousarray(
                np.asarray(Wk, np.float32).T[:, cols]).astype(bf),
            "wvT": np.ascontiguousarray(
                np.asarray(Wv, np.float32).T[:, cols]).astype(bf),
            "woT": np.ascontiguousarray(
                np.asarray(Wo, np.float32).T[cols, :]).astype(bf),
            "bc65": bc,
            "zer64": zer,
        })
    kw = dict(trace_kwargs or {})
    res = run_bass_kernel_spmd(
        _get_nc(), in_maps, list(range(N_CORES)), trace=trace, **kw
    )
    y = np.empty((B, S, D), np.float32)
    for b in range(B):
        y[b] = (res.results[2 * b]["y"].astype(np.float32)
                + res.results[2 * b + 1]["y"].astype(np.float32))
    return y, res


def kernel(x, Wq, Wk, Wv, Wo):
    y, _ = run(x, Wq, Wk, Wv, Wo)
    return y
